# revision 3
# baseline (speedup 1.0000x reference)
"""Trainium2 Bass kernel for block-causal GQA attention (B=8,S=1024,D=1024,NH=16,NKV=8,HD=64,BLK=8).

Strategy: pure data-parallel over batch (1 batch element per NeuronCore, 8 cores).
Per core, everything is computed in a transposed ("T") layout so no on-device
transposes of activations or probabilities are ever needed:
  - host feeds x[b].T;  qT/kT are projected directly as  w.T @ x.T  (lhsT=w, rhs=xT)
  - RoPE is applied in the T layout using de-interleaved head dims (weight columns
    are permuted on the host; a 32-row block swap via SBUF-SBUF DMA supplies the
    rotated partner), with the 1/sqrt(HD) score scale folded into the cos/sin tables
  - scores are computed directly transposed:  sT[j,i] = k_tile.T @ qT  with two
    heads per PE pass (GQA pair shares the kv head; row-tiled K=64 matmuls at
    tile_position (0,0)/(64,0) run concurrently)
  - softmax denominator comes free from a ones-column appended to v (M=65 matmul);
    normalization happens on the 64xS per-head output, not on the SxS probabilities
  - attn-out is produced transposed, which is exactly the lhsT layout the final
    wo projection needs.
Matmuls run in float32r (full PE rate at N>=256).
"""

import os
import sys
import types
import math
import numpy as np
from contextlib import ExitStack

for _p in ("/opt/trn_rl_repo", "/root/.axon_site/_ro/trn_rl_repo"):
    if os.path.isdir(_p) and _p not in sys.path:
        sys.path.insert(0, _p)

import concourse.bass as bass
import concourse.tile as tile
from concourse import bacc, mybir
from concourse.bass_utils import run_bass_kernel_spmd

B, S, D = 8, 1024, 1024
NH, NKV, HD = 16, 8, 64
BLK = 8
N_CORES = 8
ALPHA = (1.0 / math.sqrt(HD)) ** 0.5

f32 = mybir.dt.float32
f32r = mybir.dt.float32r
EXP = mybir.ActivationFunctionType.Exp
MULT = mybir.AluOpType.mult
ADD = mybir.AluOpType.add


def _install_profhook():
    """Register the NTFF profile hook so trace=True yields HW exec time."""
    if "antenv.axon_hooks" in sys.modules:
        return
    try:
        import antenv
        from trn_agent_boot.trn_boot import _ntff_profile_via_ctypes

        mod = types.ModuleType("antenv.axon_hooks")
        state = {"hook": _ntff_profile_via_ctypes("/opt/axon/libaxon_pjrt.so")}
        mod.set_axon_ntff_profile_hook = lambda h: state.update(hook=h)
        mod.get_axon_ntff_profile_hook = lambda: state["hook"]
        sys.modules["antenv.axon_hooks"] = mod
        antenv.axon_hooks = mod
    except Exception:
        pass


def build_program():
    nc = bacc.Bacc("TRN2", target_bir_lowering=False, debug=False, num_devices=N_CORES)

    xt_d = nc.dram_tensor("xt", [D, S], f32, kind="ExternalInput").ap()
    wq_d = nc.dram_tensor("wq", [D, NH * HD], f32, kind="ExternalInput").ap()
    wk_d = nc.dram_tensor("wk", [D, NKV * HD], f32, kind="ExternalInput").ap()
    wv_d = nc.dram_tensor("wv", [D, NKV * HD], f32, kind="ExternalInput").ap()
    wo_d = nc.dram_tensor("wo", [NH * HD, D], f32, kind="ExternalInput").ap()
    cb_d = nc.dram_tensor("cb", [128, S], f32, kind="ExternalInput").ap()
    sbc_d = nc.dram_tensor("sbc", [128, S], f32, kind="ExternalInput").ap()
    maskd_d = nc.dram_tensor("maskd", [128, 128], f32, kind="ExternalInput").ap()
    ones_d = nc.dram_tensor("ones8", [128, 8], f32, kind="ExternalInput").ap()
    out_d = nc.dram_tensor("out", [S, D], f32, kind="ExternalOutput").ap()

    with tile.TileContext(nc) as tc, ExitStack() as top:
        pers = top.enter_context(tc.tile_pool(name="pers", bufs=1))
        qT = [pers.tile([128, S], f32r, tag=f"qT{m}", name=f"qT{m}") for m in range(8)]
        kdup = [pers.tile([128, S], f32r, tag=f"kdup{t}", name=f"kdup{t}") for t in range(8)]
        vstore = [pers.tile([128, 8 * 65], f32r, tag=f"vst{s}", name=f"vst{s}") for s in range(8)]
        aoT = [pers.tile([128, S], f32r, tag=f"aoT{m}", name=f"aoT{m}") for m in range(8)]
        cb_sb = pers.tile([128, S], f32, tag="cb")
        sb_sb = pers.tile([128, S], f32, tag="sbc")
        maskd_sb = pers.tile([128, 128], f32, tag="maskd")

        nc.sync.dma_start(cb_sb[:], cb_d)
        nc.sync.dma_start(sb_sb[:], sbc_d)
        nc.sync.dma_start(maskd_sb[:], maskd_d)

        with tc.tile_pool(name="xtp", bufs=1) as xtp:
            xt_sb = [xtp.tile([128, S], f32r, tag=f"xt{d}", name=f"xt{d}") for d in range(8)]
            for d in range(8):
                nc.sync.dma_start(
                    xt_sb[d][:], xt_d[d * 128 : (d + 1) * 128, :].bitcast(f32r)
                )

            # ---- Phase 1a: v projection -> vstore (with ones col per head) ----
            with (
                tc.tile_pool(name="wvp", bufs=1) as wvp,
                tc.tile_pool(name="ps1a", bufs=2, space="PSUM") as ps1a,
            ):
                wv_sb = [wvp.tile([128, 512], f32r, tag=f"wv{d}", name=f"wv{d}") for d in range(8)]
                for d in range(8):
                    nc.sync.dma_start(
                        wv_sb[d][:], wv_d[d * 128 : (d + 1) * 128, :].bitcast(f32r)
                    )
                for s in range(8):
                    ps = ps1a.tile([128, 512], f32, tag="vps")
                    for d in range(8):
                        nc.tensor.matmul(
                            ps[:],
                            xt_sb[d][:, s * 128 : (s + 1) * 128],
                            wv_sb[d][:],
                            start=(d == 0),
                            stop=(d == 7),
                        )
                    v3 = vstore[s][:].rearrange("p (g c) -> p g c", c=65)
                    nc.vector.tensor_copy(
                        v3[:, :, 0:64], ps[:].rearrange("p (g c) -> p g c", c=64)
                    )
                    nc.sync.dma_start(
                        v3[:, :, 64:65],
                        ones_d[:].rearrange("p (g o) -> p g o", o=1).bitcast(f32r),
                    )

            # ---- Phase 1b/1c: q and k projections + RoPE ----
            with (
                tc.tile_pool(name="projp", bufs=3) as projp,
                tc.tile_pool(name="ktmpp", bufs=2) as ktmpp,
                tc.tile_pool(name="ps1b", bufs=2, space="PSUM") as ps1b,
            ):

                def rope_chunk(ps, s, dest_slice):
                    """dest_slice = rope(ps) for a [128, 512] chunk at s (0/1)."""
                    qn = projp.tile([128, 512], f32, tag="qn")
                    nc.scalar.copy(qn[:], ps[:])
                    qsw = projp.tile([128, 512], f32, tag="qsw")
                    for b0 in (0, 64):
                        nc.sync.dma_start(
                            qsw[b0 : b0 + 32, :], qn[b0 + 32 : b0 + 64, :]
                        )
                        nc.sync.dma_start(
                            qsw[b0 + 32 : b0 + 64, :], qn[b0 : b0 + 32, :]
                        )
                    t1 = projp.tile([128, 512], f32, tag="t1")
                    nc.vector.tensor_tensor(
                        t1[:], ps[:], cb_sb[:, s * 512 : (s + 1) * 512], MULT
                    )
                    t2 = projp.tile([128, 512], f32, tag="t2")
                    nc.vector.tensor_tensor(
                        t2[:], qsw[:], sb_sb[:, s * 512 : (s + 1) * 512], MULT
                    )
                    nc.vector.tensor_tensor(dest_slice, t1[:], t2[:], ADD)

                def project(m, w_d, dest_tile):
                    """dest_tile[:, :] = rope( w_d[:, m*128:+128].T @ xT )"""
                    wslice = projp.tile([128, 1024], f32r, tag="wslice")
                    nc.sync.dma_start(
                        wslice[:].rearrange("p (d c) -> p d c", c=128),
                        w_d[:, m * 128 : (m + 1) * 128]
                        .rearrange("(d p) c -> p d c", p=128)
                        .bitcast(f32r),
                    )
                    pqs = [ps1b.tile([128, 512], f32, tag=f"pq{s}", name=f"pq{s}") for s in range(2)]
                    for d in range(8):
                        lhs = wslice[:, d * 128 : (d + 1) * 128]
                        for s in range(2):
                            nc.tensor.matmul(
                                pqs[s][:],
                                lhs,
                                xt_sb[d][:, s * 512 : (s + 1) * 512],
                                start=(d == 0),
                                stop=(d == 7),
                            )
                    for s in range(2):
                        rope_chunk(
                            pqs[s], s, dest_tile[:, s * 512 : (s + 1) * 512]
                        )

                for m in range(8):
                    project(m, wq_d, qT[m])
                for m in range(4):
                    ktmp = ktmpp.tile([128, S], f32r, tag="ktmp")
                    project(m, wk_d, ktmp)
                    for half in range(2):
                        src = ktmp[half * 64 : half * 64 + 64, :]
                        kd = kdup[2 * m + half]
                        nc.sync.dma_start(kd[0:64, :], src)
                        nc.sync.dma_start(kd[64:128, :], src)

        # ---- Phase 2: attention ----
        with (
            tc.tile_pool(name="attp", bufs=3) as attp,
            tc.tile_pool(name="nrmp", bufs=2) as nrmp,
            tc.tile_pool(name="ps2s", bufs=2, space="PSUM") as ps2s,
            tc.tile_pool(name="ps2o", bufs=2, space="PSUM") as ps2o,
        ):
            for t in range(8):
                for c in range(2):
                    kjs = list(range(4 if c == 0 else 8))
                    psAB = [
                        ps2o.tile([65, 512], f32, tag=f"po{h}", name=f"po{h}") for h in range(2)
                    ]
                    for kj in kjs:
                        qlo = max(kj * 128, c * 512)
                        N = (c + 1) * 512 - qlo
                        off = qlo - c * 512
                        sAB = [
                            ps2s.tile([128, 512], f32, tag=f"sc{h}", name=f"sc{h}")
                            for h in range(2)
                        ]
                        for h in range(2):
                            nc.tensor.matmul(
                                sAB[h][:, 0:N],
                                kdup[t][h * 64 : h * 64 + 64, kj * 128 : (kj + 1) * 128],
                                qT[t][h * 64 : h * 64 + 64, qlo : qlo + N],
                                start=True,
                                stop=True,
                                tile_position=(h * 64, 0),
                            )
                        if qlo == kj * 128:
                            for h in range(2):
                                nc.vector.tensor_tensor(
                                    sAB[h][:, 0:128], sAB[h][:, 0:128], maskd_sb[:], ADD
                                )
                        vh = vstore[kj][:, t * 65 : t * 65 + 65]
                        for h in range(2):
                            p_ = attp.tile([128, 512], f32r, tag=f"p{h}")
                            nc.scalar.activation(p_[:, 0:N], sAB[h][:, 0:N], EXP)
                            nc.tensor.matmul(
                                psAB[h][0:65, off : off + N],
                                vh,
                                p_[:, 0:N],
                                start=(kj == 0),
                                stop=(kj == kjs[-1]),
                                skip_group_check=True,
                            )
                    for h in range(2):
                        ps_ = psAB[h]
                        rec = nrmp.tile([128, 512], f32, tag="rec")
                        nc.vector.reciprocal(rec[64:65, :], ps_[64:65, :])
                        nc.sync.dma_start(rec[0:1, :], rec[64:65, :])
                        bc = nrmp.tile([128, 512], f32, tag="bc")
                        nc.gpsimd.partition_broadcast(bc[0:64, :], rec[0:1, :])
                        if h == 0:
                            nc.vector.tensor_tensor(
                                aoT[t][0:64, c * 512 : (c + 1) * 512],
                                ps_[0:64, :],
                                bc[0:64, :],
                                MULT,
                            )
                        else:
                            tmpB = nrmp.tile([128, 512], f32r, tag="tmpB")
                            nc.vector.tensor_tensor(
                                tmpB[0:64, :], ps_[0:64, :], bc[0:64, :], MULT
                            )
                            nc.sync.dma_start(
                                aoT[t][64:128, c * 512 : (c + 1) * 512], tmpB[0:64, :]
                            )

        # ---- Phase 3: output projection ----
        with (
            tc.tile_pool(name="wop", bufs=1) as wop,
            tc.tile_pool(name="ostp", bufs=3) as ostp,
            tc.tile_pool(name="ps3", bufs=2, space="PSUM") as ps3,
        ):
            wo_sb = [wop.tile([128, 1024], f32r, tag=f"wo{m}", name=f"wo{m}") for m in range(8)]
            for m in range(8):
                nc.sync.dma_start(
                    wo_sb[m][:], wo_d[m * 128 : (m + 1) * 128, :].bitcast(f32r)
                )
            for st in range(8):
                po = [ps3.tile([128, 512], f32, tag=f"fo{j}", name=f"fo{j}") for j in range(2)]
                for m in range(8):
                    lhs = aoT[m][:, st * 128 : (st + 1) * 128]
                    for j in range(2):
                        nc.tensor.matmul(
                            po[j][:],
                            lhs,
                            wo_sb[m][:, j * 512 : (j + 1) * 512],
                            start=(m == 0),
                            stop=(m == 7),
                        )
                ob = ostp.tile([128, 1024], f32, tag="ob")
                for j in range(2):
                    nc.scalar.copy(ob[:, j * 512 : (j + 1) * 512], po[j][:])
                nc.sync.dma_start(out_d[st * 128 : (st + 1) * 128, :], ob[:])

    nc.compile()
    return nc


_NC = None


def _get_nc():
    global _NC
    if _NC is None:
        _NC = build_program()
    return _NC


def _host_prep(x, wq, wk, wv, wo, fcc, fcs):
    perm64 = np.concatenate([np.arange(0, 64, 2), np.arange(1, 64, 2)])
    perm_q = np.concatenate([h * 64 + perm64 for h in range(NH)])
    perm_k = np.concatenate([h * 64 + perm64 for h in range(NKV)])
    wq_p = np.ascontiguousarray(wq[:, perm_q], dtype=np.float32)
    wk_p = np.ascontiguousarray(wk[:, perm_k], dtype=np.float32)
    cb = (np.tile(fcc.T, (4, 1)) * ALPHA).astype(np.float32)
    sgn = np.where(np.arange(128) % 64 < 32, -1.0, 1.0).astype(np.float32)
    sbc = (np.tile(fcs.T, (4, 1)) * ALPHA * sgn[:, None]).astype(np.float32)
    jj = np.arange(128)[:, None] // BLK
    ii = np.arange(128)[None, :] // BLK
    maskd = np.where(ii >= jj, 0.0, -60.0).astype(np.float32)
    ones8 = np.ones((128, 8), np.float32)
    shared = {
        "wq": wq_p,
        "wk": wk_p,
        "wv": np.ascontiguousarray(wv, dtype=np.float32),
        "wo": np.ascontiguousarray(wo, dtype=np.float32),
        "cb": cb,
        "sbc": sbc,
        "maskd": maskd,
        "ones8": ones8,
    }
    in_maps = []
    for b in range(B):
        m = dict(shared)
        m["xt"] = np.ascontiguousarray(x[b].T, dtype=np.float32)
        in_maps.append(m)
    return in_maps


def kernel(x, wq, wk, wv, wo, freqs_cis_cos, freqs_cis_sin, start_pos=0):
    _install_profhook()
    x = np.asarray(x, dtype=np.float32)
    in_maps = _host_prep(
        x,
        np.asarray(wq, dtype=np.float32),
        np.asarray(wk, dtype=np.float32),
        np.asarray(wv, dtype=np.float32),
        np.asarray(wo, dtype=np.float32),
        np.asarray(freqs_cis_cos, dtype=np.float32),
        np.asarray(freqs_cis_sin, dtype=np.float32),
    )
    nc = _get_nc()
    trace = bool(int(os.environ.get("KERNEL_TRACE", "0")))
    res = run_bass_kernel_spmd(
        nc, in_maps, core_ids=list(range(N_CORES)), trace=trace
    )
    if trace:
        kernel.last_exec_time_ns = res.exec_time_ns
        kernel.last_result = res
    out = np.stack([res.results[c]["out"] for c in range(N_CORES)])
    return out


kernel.last_exec_time_ns = None
kernel.last_result = None


# revision 10
# speedup vs baseline: 1.2095x; 1.2095x over previous
"""Trainium2 Bass kernel for block-causal GQA attention (B=8,S=1024,D=1024,NH=16,NKV=8,HD=64,BLK=8).

Strategy: pure data-parallel over batch (1 batch element per NeuronCore, 8 cores).
Per core, everything is computed in a transposed ("T") layout so no on-device
transposes of activations or probabilities are ever needed:
  - host feeds x[b].T;  qT/kT are projected directly as  w.T @ x.T  (lhsT=w, rhs=xT)
  - RoPE is applied in the T layout using de-interleaved head dims (weight columns
    are permuted on the host; a 32-row block swap via SBUF-SBUF DMA supplies the
    rotated partner), with the 1/sqrt(HD) score scale folded into the cos/sin tables
  - scores are computed directly transposed:  sT[j,i] = k_tile.T @ qT  with two
    heads per PE pass (GQA pair shares the kv head; row-tiled K=64 matmuls at
    tile_position (0,0)/(64,0) run concurrently into the two banks of one
    2-bank PSUM tile, so softmax exp / reciprocal / broadcast run once per pair)
  - softmax denominator comes free from a ones-column appended to v (M=65 matmul);
    normalization happens on the 64xS per-head output, not on the SxS probabilities
  - attn-out is produced transposed, which is exactly the lhsT layout the final
    wo projection needs.
Matmuls run in float32r (full PE rate at N>=256).
"""

import os
import sys
import types
import math
import numpy as np
from contextlib import ExitStack

for _p in ("/opt/trn_rl_repo", "/root/.axon_site/_ro/trn_rl_repo"):
    if os.path.isdir(_p) and _p not in sys.path:
        sys.path.insert(0, _p)

import concourse.bass as bass
import concourse.tile as tile
from concourse import bacc, mybir
from concourse.bass_utils import run_bass_kernel_spmd

B, S, D = 8, 1024, 1024
NH, NKV, HD = 16, 8, 64
BLK = 8
N_CORES = 8
ALPHA = (1.0 / math.sqrt(HD)) ** 0.5

f32 = mybir.dt.float32
f32r = mybir.dt.float32r
EXP = mybir.ActivationFunctionType.Exp
MULT = mybir.AluOpType.mult
ADD = mybir.AluOpType.add


def _install_profhook():
    """Register the NTFF profile hook so trace=True yields HW exec time."""
    if "antenv.axon_hooks" in sys.modules:
        return
    try:
        import antenv
        from trn_agent_boot.trn_boot import _ntff_profile_via_ctypes

        mod = types.ModuleType("antenv.axon_hooks")
        state = {"hook": _ntff_profile_via_ctypes("/opt/axon/libaxon_pjrt.so")}
        mod.set_axon_ntff_profile_hook = lambda h: state.update(hook=h)
        mod.get_axon_ntff_profile_hook = lambda: state["hook"]
        sys.modules["antenv.axon_hooks"] = mod
        antenv.axon_hooks = mod
    except Exception:
        pass


def build_program():
    nc = bacc.Bacc("TRN2", target_bir_lowering=False, debug=False, num_devices=N_CORES)

    xt_d = nc.dram_tensor("xt", [D, S], f32, kind="ExternalInput").ap()
    wq_d = nc.dram_tensor("wq", [D, NH * HD], f32, kind="ExternalInput").ap()
    wk_d = nc.dram_tensor("wk", [D, NKV * HD], f32, kind="ExternalInput").ap()
    wv_d = nc.dram_tensor("wv", [D, NKV * HD], f32, kind="ExternalInput").ap()
    wo_d = nc.dram_tensor("wo", [NH * HD, D], f32, kind="ExternalInput").ap()
    cb_d = nc.dram_tensor("cb", [128, S], f32, kind="ExternalInput").ap()
    sbc_d = nc.dram_tensor("sbc", [128, S], f32, kind="ExternalInput").ap()
    maskd_d = nc.dram_tensor("maskd", [128, 128], f32, kind="ExternalInput").ap()
    ones_d = nc.dram_tensor("ones8", [128, 8], f32, kind="ExternalInput").ap()
    out_d = nc.dram_tensor("out", [S, D], f32, kind="ExternalOutput").ap()

    with tile.TileContext(nc) as tc, ExitStack() as top:
        pers = top.enter_context(tc.tile_pool(name="pers", bufs=1))
        qT = [pers.tile([128, S], f32r, tag=f"qT{m}", name=f"qT{m}") for m in range(8)]
        kdup = [pers.tile([128, S], f32r, tag=f"kdup{t}", name=f"kdup{t}") for t in range(8)]
        vstore = [pers.tile([128, 8 * 65], f32r, tag=f"vst{s}", name=f"vst{s}") for s in range(8)]
        cb_sb = pers.tile([128, S], f32, tag="cb")
        sb_sb = pers.tile([128, S], f32, tag="sbc")
        maskb_sb = pers.tile([128, 128], f32r, tag="maskb")

        nc.sync.dma_start(cb_sb[:], cb_d)
        nc.sync.dma_start(sb_sb[:], sbc_d)
        nc.sync.dma_start(maskb_sb[:], maskd_d.bitcast(f32r))

        with tc.tile_pool(name="xtp", bufs=1) as xtp:
            xt_sb = [xtp.tile([128, S], f32r, tag=f"xt{d}", name=f"xt{d}") for d in range(8)]
            for d in range(8):
                nc.sync.dma_start(
                    xt_sb[d][:], xt_d[d * 128 : (d + 1) * 128, :].bitcast(f32r)
                )

            # ---- Phase 1a: v projection -> vstore (with ones col per head) ----
            with (
                tc.tile_pool(name="wvp", bufs=1) as wvp,
                tc.tile_pool(name="ps1a", bufs=2, space="PSUM") as ps1a,
            ):
                wv_sb = [wvp.tile([128, 512], f32r, tag=f"wv{d}", name=f"wv{d}") for d in range(8)]
                for d in range(8):
                    nc.sync.dma_start(
                        wv_sb[d][:], wv_d[d * 128 : (d + 1) * 128, :].bitcast(f32r)
                    )
                for s in range(8):
                    ps = ps1a.tile([128, 512], f32, tag="vps")
                    for d in range(8):
                        nc.tensor.matmul(
                            ps[:],
                            xt_sb[d][:, s * 128 : (s + 1) * 128],
                            wv_sb[d][:],
                            start=(d == 0),
                            stop=(d == 7),
                        )
                    v3 = vstore[s][:].rearrange("p (g c) -> p g c", c=65)
                    nc.vector.tensor_copy(
                        v3[:, :, 0:64], ps[:].rearrange("p (g c) -> p g c", c=64)
                    )
                    nc.sync.dma_start(
                        v3[:, :, 64:65],
                        ones_d[:].rearrange("p (g o) -> p g o", o=1).bitcast(f32r),
                    )

            # ---- Phase 1b/1c: k then q projections + RoPE ----
            with (
                tc.tile_pool(name="projp", bufs=2) as projp,
                tc.tile_pool(name="ktmpp", bufs=2) as ktmpp,
                tc.tile_pool(name="ps1b", bufs=2, space="PSUM") as ps1b,
            ):

                def project(m, w_d, dest_tile):
                    """dest_tile[:, :] = rope( w_d[:, m*128:+128].T @ xT )"""
                    wslice = projp.tile([128, 1024], f32r, tag="wslice")
                    nc.sync.dma_start(
                        wslice[:].rearrange("p (d c) -> p d c", c=128),
                        w_d[:, m * 128 : (m + 1) * 128]
                        .rearrange("(d p) c -> p d c", p=128)
                        .bitcast(f32r),
                    )
                    pqs = [
                        ps1b.tile([128, 512], f32, tag=f"pq{s}", name=f"pq{s}")
                        for s in range(2)
                    ]
                    for d in range(8):
                        lhs = wslice[:, d * 128 : (d + 1) * 128]
                        for s in range(2):
                            nc.tensor.matmul(
                                pqs[s][:],
                                lhs,
                                xt_sb[d][:, s * 512 : (s + 1) * 512],
                                start=(d == 0),
                                stop=(d == 7),
                            )
                    # rope on the full [128, 1024] row block
                    qn = projp.tile([128, S], f32, tag="qn")
                    for s in range(2):
                        nc.scalar.copy(qn[:, s * 512 : (s + 1) * 512], pqs[s][:])
                    qsw = projp.tile([128, S], f32, tag="qsw")
                    for b0 in (0, 64):
                        nc.sync.dma_start(
                            qsw[b0 : b0 + 32, :], qn[b0 + 32 : b0 + 64, :]
                        )
                        nc.sync.dma_start(
                            qsw[b0 + 32 : b0 + 64, :], qn[b0 : b0 + 32, :]
                        )
                    nc.vector.tensor_tensor(qsw[:], qsw[:], sb_sb[:], MULT)
                    nc.vector.tensor_tensor(qn[:], qn[:], cb_sb[:], MULT)
                    nc.vector.tensor_tensor(dest_tile[:], qn[:], qsw[:], ADD)

                for m in range(4):
                    ktmp = ktmpp.tile([128, S], f32r, tag="ktmp")
                    project(m, wk_d, ktmp)
                    for half in range(2):
                        src = ktmp[half * 64 : half * 64 + 64, :]
                        kd = kdup[2 * m + half]
                        nc.sync.dma_start(kd[0:64, :], src)
                        nc.sync.dma_start(kd[64:128, :], src)
                for m in range(8):
                    project(m, wq_d, qT[m])

        # ---- Phase 2: attention ----
        aotp = top.enter_context(tc.tile_pool(name="aotp", bufs=1))
        aoT = [aotp.tile([128, S], f32r, tag=f"aoT{m}", name=f"aoT{m}") for m in range(8)]
        with (
            tc.tile_pool(name="attp", bufs=3) as attp,
            tc.tile_pool(name="nrmp", bufs=2) as nrmp,
            tc.tile_pool(name="ps2s", bufs=2, space="PSUM") as ps2s,
            tc.tile_pool(name="ps2o", bufs=2, space="PSUM") as ps2o,
        ):
            for t in range(8):
                for c in range(2):
                    kjs = list(range(4 if c == 0 else 8))
                    po = ps2o.tile([65, 1024], f32, tag="po")  # A bank | B bank
                    for kj in kjs:
                        qlo = max(kj * 128, c * 512)
                        N = (c + 1) * 512 - qlo
                        off = qlo - c * 512
                        sc = ps2s.tile([128, 1024], f32, tag="sc")  # A | B
                        for h in range(2):
                            nc.tensor.matmul(
                                sc[:, h * 512 : h * 512 + N],
                                kdup[t][h * 64 : h * 64 + 64, kj * 128 : (kj + 1) * 128],
                                qT[t][h * 64 : h * 64 + 64, qlo : qlo + N],
                                start=True,
                                stop=True,
                                tile_position=(h * 64, 0),
                            )
                        p_ = attp.tile([128, 1024], f32r, tag="p")
                        p3 = p_[:].rearrange("p (h n) -> p h n", h=2)
                        s3 = sc[:].rearrange("p (h n) -> p h n", h=2)
                        nc.scalar.activation(p3[:, :, 0:N], s3[:, :, 0:N], EXP)
                        if qlo == kj * 128:
                            for h in range(2):
                                nc.vector.tensor_tensor(
                                    p_[:, h * 512 : h * 512 + 128],
                                    p_[:, h * 512 : h * 512 + 128],
                                    maskb_sb[:],
                                    MULT,
                                )
                        vh = vstore[kj][:, t * 65 : t * 65 + 65]
                        for h in range(2):
                            nc.tensor.matmul(
                                po[0:65, h * 512 + off : h * 512 + off + N],
                                vh,
                                p_[:, h * 512 : h * 512 + N],
                                start=(kj == 0),
                                stop=(kj == kjs[-1]),
                                skip_group_check=True,
                            )
                    rec = nrmp.tile([128, S], f32, tag="rec")
                    nc.scalar.copy(rec[64:65, :], po[64:65, :])
                    nc.sync.dma_start(rec[0:1, :], rec[64:65, :])
                    nc.vector.reciprocal_approx_fast(rec[0:1, :], rec[0:1, :])
                    bc = nrmp.tile([128, S], f32, tag="bc")
                    nc.gpsimd.partition_broadcast(bc[0:64, :], rec[0:1, :])
                    nc.vector.tensor_tensor(
                        aoT[t][0:64, c * 512 : (c + 1) * 512],
                        po[0:64, 0:512],
                        bc[0:64, 0:512],
                        MULT,
                    )
                    tmpB = nrmp.tile([128, 512], f32r, tag="tmpB")
                    nc.vector.tensor_tensor(
                        tmpB[0:64, :], po[0:64, 512:1024], bc[0:64, 512:1024], MULT
                    )
                    nc.sync.dma_start(
                        aoT[t][64:128, c * 512 : (c + 1) * 512], tmpB[0:64, :]
                    )

        # ---- Phase 3: output projection ----
        with (
            tc.tile_pool(name="wop", bufs=1) as wop,
            tc.tile_pool(name="ostp", bufs=3) as ostp,
            tc.tile_pool(name="ps3", bufs=2, space="PSUM") as ps3,
        ):
            wo_sb = [wop.tile([128, 1024], f32r, tag=f"wo{m}", name=f"wo{m}") for m in range(8)]
            for m in range(8):
                nc.sync.dma_start(
                    wo_sb[m][:], wo_d[m * 128 : (m + 1) * 128, :].bitcast(f32r)
                )
            for st in range(8):
                po = [ps3.tile([128, 512], f32, tag=f"fo{j}", name=f"fo{j}") for j in range(2)]
                for m in range(8):
                    lhs = aoT[m][:, st * 128 : (st + 1) * 128]
                    for j in range(2):
                        nc.tensor.matmul(
                            po[j][:],
                            lhs,
                            wo_sb[m][:, j * 512 : (j + 1) * 512],
                            start=(m == 0),
                            stop=(m == 7),
                        )
                ob = ostp.tile([128, 1024], f32, tag="ob")
                for j in range(2):
                    nc.scalar.copy(ob[:, j * 512 : (j + 1) * 512], po[j][:])
                nc.sync.dma_start(out_d[st * 128 : (st + 1) * 128, :], ob[:])

    nc.compile()
    return nc


_NC = None


def _get_nc():
    global _NC
    if _NC is None:
        _NC = build_program()
    return _NC


def _host_prep(x, wq, wk, wv, wo, fcc, fcs):
    perm64 = np.concatenate([np.arange(0, 64, 2), np.arange(1, 64, 2)])
    perm_q = np.concatenate([h * 64 + perm64 for h in range(NH)])
    perm_k = np.concatenate([h * 64 + perm64 for h in range(NKV)])
    wq_p = np.ascontiguousarray(wq[:, perm_q], dtype=np.float32)
    wk_p = np.ascontiguousarray(wk[:, perm_k], dtype=np.float32)
    cb = (np.tile(fcc.T, (4, 1)) * ALPHA).astype(np.float32)
    sgn = np.where(np.arange(128) % 64 < 32, -1.0, 1.0).astype(np.float32)
    sbc = (np.tile(fcs.T, (4, 1)) * ALPHA * sgn[:, None]).astype(np.float32)
    jj = np.arange(128)[:, None] // BLK
    ii = np.arange(128)[None, :] // BLK
    maskd = np.where(ii >= jj, 1.0, 0.0).astype(np.float32)
    ones8 = np.ones((128, 8), np.float32)
    shared = {
        "wq": wq_p,
        "wk": wk_p,
        "wv": np.ascontiguousarray(wv, dtype=np.float32),
        "wo": np.ascontiguousarray(wo, dtype=np.float32),
        "cb": cb,
        "sbc": sbc,
        "maskd": maskd,
        "ones8": ones8,
    }
    in_maps = []
    for b in range(B):
        m = dict(shared)
        m["xt"] = np.ascontiguousarray(x[b].T, dtype=np.float32)
        in_maps.append(m)
    return in_maps


def kernel(x, wq, wk, wv, wo, freqs_cis_cos, freqs_cis_sin, start_pos=0):
    _install_profhook()
    x = np.asarray(x, dtype=np.float32)
    in_maps = _host_prep(
        x,
        np.asarray(wq, dtype=np.float32),
        np.asarray(wk, dtype=np.float32),
        np.asarray(wv, dtype=np.float32),
        np.asarray(wo, dtype=np.float32),
        np.asarray(freqs_cis_cos, dtype=np.float32),
        np.asarray(freqs_cis_sin, dtype=np.float32),
    )
    nc = _get_nc()
    trace = bool(int(os.environ.get("KERNEL_TRACE", "0")))
    res = run_bass_kernel_spmd(
        nc, in_maps, core_ids=list(range(N_CORES)), trace=trace
    )
    if trace:
        kernel.last_exec_time_ns = res.exec_time_ns
        kernel.last_result = res
    out = np.stack([res.results[c]["out"] for c in range(N_CORES)])
    return out


kernel.last_exec_time_ns = None
kernel.last_result = None


# revision 11
# speedup vs baseline: 1.2758x; 1.0548x over previous
"""Trainium2 Bass kernel for block-causal GQA attention (B=8,S=1024,D=1024,NH=16,NKV=8,HD=64,BLK=8).

Strategy: pure data-parallel over batch (1 batch element per NeuronCore, 8 cores).
Per core, everything is computed in a transposed ("T") layout so no on-device
transposes of activations or probabilities are ever needed:
  - host feeds x[b].T;  qT/kT are projected directly as  w.T @ x.T  (lhsT=w, rhs=xT)
  - RoPE is applied in the T layout using de-interleaved head dims (weight columns
    are permuted on the host; a 32-row block swap via SBUF-SBUF DMA supplies the
    rotated partner), with the 1/sqrt(HD) score scale folded into the cos/sin tables
  - scores are computed directly transposed:  sT[j,i] = k_tile.T @ qT  with two
    heads per PE pass (GQA pair shares the kv head; row-tiled K=64 matmuls at
    tile_position (0,0)/(64,0) run concurrently into the two banks of one
    2-bank PSUM tile, so softmax exp / reciprocal / broadcast run once per pair)
  - softmax denominator comes free from a ones-column appended to v (M=65 matmul);
    normalization happens on the 64xS per-head output, not on the SxS probabilities
  - attn-out is produced transposed, which is exactly the lhsT layout the final
    wo projection needs.
Matmuls run in float32r (full PE rate at N>=256).
"""

import os
import sys
import types
import math
import numpy as np
from contextlib import ExitStack

for _p in ("/opt/trn_rl_repo", "/root/.axon_site/_ro/trn_rl_repo"):
    if os.path.isdir(_p) and _p not in sys.path:
        sys.path.insert(0, _p)

import concourse.bass as bass
import concourse.tile as tile
from concourse import bacc, mybir
from concourse.bass_utils import run_bass_kernel_spmd

B, S, D = 8, 1024, 1024
NH, NKV, HD = 16, 8, 64
BLK = 8
N_CORES = 8
ALPHA = (1.0 / math.sqrt(HD)) ** 0.5

f32 = mybir.dt.float32
f32r = mybir.dt.float32r
EXP = mybir.ActivationFunctionType.Exp
MULT = mybir.AluOpType.mult
ADD = mybir.AluOpType.add


def _install_profhook():
    """Register the NTFF profile hook so trace=True yields HW exec time."""
    if "antenv.axon_hooks" in sys.modules:
        return
    try:
        import antenv
        from trn_agent_boot.trn_boot import _ntff_profile_via_ctypes

        mod = types.ModuleType("antenv.axon_hooks")
        state = {"hook": _ntff_profile_via_ctypes("/opt/axon/libaxon_pjrt.so")}
        mod.set_axon_ntff_profile_hook = lambda h: state.update(hook=h)
        mod.get_axon_ntff_profile_hook = lambda: state["hook"]
        sys.modules["antenv.axon_hooks"] = mod
        antenv.axon_hooks = mod
    except Exception:
        pass


def build_program():
    nc = bacc.Bacc("TRN2", target_bir_lowering=False, debug=False, num_devices=N_CORES)

    xt_d = nc.dram_tensor("xt", [D, S], f32, kind="ExternalInput").ap()
    wq_d = nc.dram_tensor("wq", [D, NH * HD], f32, kind="ExternalInput").ap()
    wk_d = nc.dram_tensor("wk", [D, NKV * HD], f32, kind="ExternalInput").ap()
    wv_d = nc.dram_tensor("wv", [D, NKV * HD], f32, kind="ExternalInput").ap()
    wo_d = nc.dram_tensor("wo", [NH * HD, D], f32, kind="ExternalInput").ap()
    cb_d = nc.dram_tensor("cb", [128, S], f32, kind="ExternalInput").ap()
    sbc_d = nc.dram_tensor("sbc", [128, S], f32, kind="ExternalInput").ap()
    maskd_d = nc.dram_tensor("maskd", [128, 128], f32, kind="ExternalInput").ap()
    ones_d = nc.dram_tensor("ones8", [128, 8], f32, kind="ExternalInput").ap()
    out_d = nc.dram_tensor("out", [S, D], f32, kind="ExternalOutput").ap()

    with tile.TileContext(nc) as tc, ExitStack() as top:
        pers = top.enter_context(tc.tile_pool(name="pers", bufs=1))
        qT = [pers.tile([128, S], f32r, tag=f"qT{m}", name=f"qT{m}") for m in range(8)]
        kdup = [pers.tile([128, S], f32r, tag=f"kdup{t}", name=f"kdup{t}") for t in range(8)]
        vstore = [pers.tile([128, 8 * 65], f32r, tag=f"vst{s}", name=f"vst{s}") for s in range(8)]
        cb_sb = pers.tile([128, S], f32, tag="cb")
        sb_sb = pers.tile([128, S], f32, tag="sbc")
        maskb_sb = pers.tile([128, 128], f32r, tag="maskb")

        nc.sync.dma_start(cb_sb[:], cb_d)
        nc.sync.dma_start(sb_sb[:], sbc_d)
        nc.sync.dma_start(maskb_sb[:], maskd_d.bitcast(f32r))

        with tc.tile_pool(name="xtp", bufs=1) as xtp:
            xt_sb = [xtp.tile([128, S], f32r, tag=f"xt{d}", name=f"xt{d}") for d in range(8)]
            for d in range(8):
                nc.sync.dma_start(
                    xt_sb[d][:], xt_d[d * 128 : (d + 1) * 128, :].bitcast(f32r)
                )

            # ---- Phase 1a: v projection -> vstore (with ones col per head) ----
            with (
                tc.tile_pool(name="wvp", bufs=1) as wvp,
                tc.tile_pool(name="ps1a", bufs=2, space="PSUM") as ps1a,
            ):
                wv_sb = [wvp.tile([128, 512], f32r, tag=f"wv{d}", name=f"wv{d}") for d in range(8)]
                for d in range(8):
                    nc.sync.dma_start(
                        wv_sb[d][:], wv_d[d * 128 : (d + 1) * 128, :].bitcast(f32r)
                    )
                for s in range(8):
                    ps = ps1a.tile([128, 512], f32, tag="vps")
                    for d in range(8):
                        nc.tensor.matmul(
                            ps[:],
                            xt_sb[d][:, s * 128 : (s + 1) * 128],
                            wv_sb[d][:],
                            start=(d == 0),
                            stop=(d == 7),
                        )
                    v3 = vstore[s][:].rearrange("p (g c) -> p g c", c=65)
                    nc.vector.tensor_copy(
                        v3[:, :, 0:64], ps[:].rearrange("p (g c) -> p g c", c=64)
                    )
                    nc.sync.dma_start(
                        v3[:, :, 64:65],
                        ones_d[:].rearrange("p (g o) -> p g o", o=1).bitcast(f32r),
                    )

            # ---- Phase 1b/1c: k then q projections + RoPE ----
            with (
                tc.tile_pool(name="projp", bufs=3) as projp,
                tc.tile_pool(name="ktmpp", bufs=2) as ktmpp,
                tc.tile_pool(name="ps1b", bufs=3, space="PSUM") as ps1b,
            ):

                def project(m, w_d, dest_tile):
                    """dest_tile[:, :] = rope( w_d[:, m*128:+128].T @ xT )"""
                    wslice = projp.tile([128, 1024], f32r, tag="wslice")
                    nc.sync.dma_start(
                        wslice[:].rearrange("p (d c) -> p d c", c=128),
                        w_d[:, m * 128 : (m + 1) * 128]
                        .rearrange("(d p) c -> p d c", p=128)
                        .bitcast(f32r),
                    )
                    pqs = [
                        ps1b.tile([128, 512], f32, tag=f"pq{s}", name=f"pq{s}")
                        for s in range(2)
                    ]
                    for d in range(8):
                        lhs = wslice[:, d * 128 : (d + 1) * 128]
                        for s in range(2):
                            nc.tensor.matmul(
                                pqs[s][:],
                                lhs,
                                xt_sb[d][:, s * 512 : (s + 1) * 512],
                                start=(d == 0),
                                stop=(d == 7),
                            )
                    # rope on the full [128, 1024] row block
                    qn = projp.tile([128, S], f32, tag="qn")
                    for s in range(2):
                        nc.scalar.copy(qn[:, s * 512 : (s + 1) * 512], pqs[s][:])
                    qsw = projp.tile([128, S], f32, tag="qsw")
                    for b0 in (0, 64):
                        nc.sync.dma_start(
                            qsw[b0 : b0 + 32, :], qn[b0 + 32 : b0 + 64, :]
                        )
                        nc.sync.dma_start(
                            qsw[b0 + 32 : b0 + 64, :], qn[b0 : b0 + 32, :]
                        )
                    nc.vector.tensor_tensor(qsw[:], qsw[:], sb_sb[:], MULT)
                    nc.vector.tensor_tensor(qn[:], qn[:], cb_sb[:], MULT)
                    nc.vector.tensor_tensor(dest_tile[:], qn[:], qsw[:], ADD)

                for m in range(4):
                    ktmp = ktmpp.tile([128, S], f32r, tag="ktmp")
                    project(m, wk_d, ktmp)
                    for half in range(2):
                        src = ktmp[half * 64 : half * 64 + 64, :]
                        kd = kdup[2 * m + half]
                        nc.sync.dma_start(kd[0:64, :], src)
                        nc.sync.dma_start(kd[64:128, :], src)
                for m in range(8):
                    project(m, wq_d, qT[m])

        # ---- Phase 2: attention ----
        aotp = top.enter_context(tc.tile_pool(name="aotp", bufs=1))
        aoT = [aotp.tile([128, S], f32r, tag=f"aoT{m}", name=f"aoT{m}") for m in range(8)]
        with (
            tc.tile_pool(name="attp", bufs=3) as attp,
            tc.tile_pool(name="nrmp", bufs=2) as nrmp,
            tc.tile_pool(name="ps2s", bufs=2, space="PSUM") as ps2s,
            tc.tile_pool(name="ps2o", bufs=2, space="PSUM") as ps2o,
        ):
            for t in range(8):
                for c in range(2):
                    kjs = list(range(4 if c == 0 else 8))
                    po = ps2o.tile([65, 1024], f32, tag="po")  # A bank | B bank

                    def scores_exp(kj):
                        qlo = max(kj * 128, c * 512)
                        N = (c + 1) * 512 - qlo
                        sc = ps2s.tile([128, 1024], f32, tag="sc", name="sc")
                        for h in range(2):
                            nc.tensor.matmul(
                                sc[:, h * 512 : h * 512 + N],
                                kdup[t][h * 64 : h * 64 + 64, kj * 128 : (kj + 1) * 128],
                                qT[t][h * 64 : h * 64 + 64, qlo : qlo + N],
                                start=True,
                                stop=True,
                                tile_position=(h * 64, 0),
                            )
                        p_ = attp.tile([128, 1024], f32r, tag="p", name="p")
                        p3 = p_[:].rearrange("p (h n) -> p h n", h=2)
                        s3 = sc[:].rearrange("p (h n) -> p h n", h=2)
                        nc.scalar.activation(p3[:, :, 0:N], s3[:, :, 0:N], EXP)
                        if qlo == kj * 128:
                            for h in range(2):
                                nc.vector.tensor_tensor(
                                    p_[:, h * 512 : h * 512 + 128],
                                    p_[:, h * 512 : h * 512 + 128],
                                    maskb_sb[:],
                                    MULT,
                                )
                        return p_, qlo, N

                    def vmms(kj, p_, qlo, N):
                        off = qlo - c * 512
                        vh = vstore[kj][:, t * 65 : t * 65 + 65]
                        for h in range(2):
                            nc.tensor.matmul(
                                po[0:65, h * 512 + off : h * 512 + off + N],
                                vh,
                                p_[:, h * 512 : h * 512 + N],
                                start=(kj == 0),
                                stop=(kj == kjs[-1]),
                                skip_group_check=True,
                            )

                    pending = None
                    for kj in kjs:
                        se = scores_exp(kj)
                        if pending is not None:
                            vmms(*pending)
                        pending = (kj, *se) and (kj, se[0], se[1], se[2])
                    vmms(*pending)
                    rec = nrmp.tile([128, S], f32, tag="rec")
                    nc.scalar.copy(rec[64:65, :], po[64:65, :])
                    nc.sync.dma_start(rec[0:1, :], rec[64:65, :])
                    nc.vector.reciprocal_approx_fast(rec[0:1, :], rec[0:1, :])
                    bc = nrmp.tile([128, S], f32, tag="bc")
                    nc.gpsimd.partition_broadcast(bc[0:64, :], rec[0:1, :])
                    nc.vector.tensor_tensor(
                        aoT[t][0:64, c * 512 : (c + 1) * 512],
                        po[0:64, 0:512],
                        bc[0:64, 0:512],
                        MULT,
                    )
                    tmpB = nrmp.tile([128, 512], f32r, tag="tmpB")
                    nc.vector.tensor_tensor(
                        tmpB[0:64, :], po[0:64, 512:1024], bc[0:64, 512:1024], MULT
                    )
                    nc.sync.dma_start(
                        aoT[t][64:128, c * 512 : (c + 1) * 512], tmpB[0:64, :]
                    )

        # ---- Phase 3: output projection ----
        with (
            tc.tile_pool(name="wop", bufs=1) as wop,
            tc.tile_pool(name="ostp", bufs=3) as ostp,
            tc.tile_pool(name="ps3", bufs=2, space="PSUM") as ps3,
        ):
            wo_sb = [wop.tile([128, 1024], f32r, tag=f"wo{m}", name=f"wo{m}") for m in range(8)]
            for m in range(8):
                nc.sync.dma_start(
                    wo_sb[m][:], wo_d[m * 128 : (m + 1) * 128, :].bitcast(f32r)
                )
            for st in range(8):
                po = [ps3.tile([128, 512], f32, tag=f"fo{j}", name=f"fo{j}") for j in range(2)]
                for m in range(8):
                    lhs = aoT[m][:, st * 128 : (st + 1) * 128]
                    for j in range(2):
                        nc.tensor.matmul(
                            po[j][:],
                            lhs,
                            wo_sb[m][:, j * 512 : (j + 1) * 512],
                            start=(m == 0),
                            stop=(m == 7),
                        )
                ob = ostp.tile([128, 1024], f32, tag="ob")
                for j in range(2):
                    nc.scalar.copy(ob[:, j * 512 : (j + 1) * 512], po[j][:])
                nc.sync.dma_start(out_d[st * 128 : (st + 1) * 128, :], ob[:])

    nc.compile()
    return nc


_NC = None


def _get_nc():
    global _NC
    if _NC is None:
        _NC = build_program()
    return _NC


def _host_prep(x, wq, wk, wv, wo, fcc, fcs):
    perm64 = np.concatenate([np.arange(0, 64, 2), np.arange(1, 64, 2)])
    perm_q = np.concatenate([h * 64 + perm64 for h in range(NH)])
    perm_k = np.concatenate([h * 64 + perm64 for h in range(NKV)])
    wq_p = np.ascontiguousarray(wq[:, perm_q], dtype=np.float32)
    wk_p = np.ascontiguousarray(wk[:, perm_k], dtype=np.float32)
    cb = (np.tile(fcc.T, (4, 1)) * ALPHA).astype(np.float32)
    sgn = np.where(np.arange(128) % 64 < 32, -1.0, 1.0).astype(np.float32)
    sbc = (np.tile(fcs.T, (4, 1)) * ALPHA * sgn[:, None]).astype(np.float32)
    jj = np.arange(128)[:, None] // BLK
    ii = np.arange(128)[None, :] // BLK
    maskd = np.where(ii >= jj, 1.0, 0.0).astype(np.float32)
    ones8 = np.ones((128, 8), np.float32)
    shared = {
        "wq": wq_p,
        "wk": wk_p,
        "wv": np.ascontiguousarray(wv, dtype=np.float32),
        "wo": np.ascontiguousarray(wo, dtype=np.float32),
        "cb": cb,
        "sbc": sbc,
        "maskd": maskd,
        "ones8": ones8,
    }
    in_maps = []
    for b in range(B):
        m = dict(shared)
        m["xt"] = np.ascontiguousarray(x[b].T, dtype=np.float32)
        in_maps.append(m)
    return in_maps


def kernel(x, wq, wk, wv, wo, freqs_cis_cos, freqs_cis_sin, start_pos=0):
    _install_profhook()
    x = np.asarray(x, dtype=np.float32)
    in_maps = _host_prep(
        x,
        np.asarray(wq, dtype=np.float32),
        np.asarray(wk, dtype=np.float32),
        np.asarray(wv, dtype=np.float32),
        np.asarray(wo, dtype=np.float32),
        np.asarray(freqs_cis_cos, dtype=np.float32),
        np.asarray(freqs_cis_sin, dtype=np.float32),
    )
    nc = _get_nc()
    trace = bool(int(os.environ.get("KERNEL_TRACE", "0")))
    res = run_bass_kernel_spmd(
        nc, in_maps, core_ids=list(range(N_CORES)), trace=trace
    )
    if trace:
        kernel.last_exec_time_ns = res.exec_time_ns
        kernel.last_result = res
    out = np.stack([res.results[c]["out"] for c in range(N_CORES)])
    return out


kernel.last_exec_time_ns = None
kernel.last_result = None


# revision 12
# speedup vs baseline: 1.4014x; 1.0985x over previous
"""Trainium2 Bass kernel for block-causal GQA attention (B=8,S=1024,D=1024,NH=16,NKV=8,HD=64,BLK=8).

Strategy: pure data-parallel over batch (1 batch element per NeuronCore, 8 cores).
Per core, everything is computed in a transposed ("T") layout so no on-device
transposes of activations or probabilities are ever needed:
  - host feeds x[b].T;  qT/kT are projected directly as  w.T @ x.T  (lhsT=w, rhs=xT)
  - RoPE is applied in the T layout using de-interleaved head dims (weight columns
    are permuted on the host; a 32-row block swap via SBUF-SBUF DMA supplies the
    rotated partner), with the 1/sqrt(HD) score scale folded into the cos/sin tables
  - scores are computed directly transposed:  sT[j,i] = k_tile.T @ qT  with two
    heads per PE pass (GQA pair shares the kv head; row-tiled K=64 matmuls at
    tile_position (0,0)/(64,0) run concurrently into the two banks of one
    2-bank PSUM tile, so softmax exp / reciprocal / broadcast run once per pair)
  - softmax denominator comes free from a ones-column appended to v (M=65 matmul);
    normalization happens on the 64xS per-head output, not on the SxS probabilities
  - attn-out is produced transposed, which is exactly the lhsT layout the final
    wo projection needs.
Matmuls run in float32r (full PE rate at N>=256).
"""

import os
import sys
import types
import math
import numpy as np
from contextlib import ExitStack

for _p in ("/opt/trn_rl_repo", "/root/.axon_site/_ro/trn_rl_repo"):
    if os.path.isdir(_p) and _p not in sys.path:
        sys.path.insert(0, _p)

import concourse.bass as bass
import concourse.tile as tile
from concourse import bacc, mybir
from concourse.bass_utils import run_bass_kernel_spmd

B, S, D = 8, 1024, 1024
NH, NKV, HD = 16, 8, 64
BLK = 8
N_CORES = 8
ALPHA = (1.0 / math.sqrt(HD)) ** 0.5

f32 = mybir.dt.float32
f32r = mybir.dt.float32r
f16 = mybir.dt.float16
MM_DT = f16 if os.environ.get("KERNEL_MM_DT", "f16") == "f16" else f32r
MM_NP = np.float16 if MM_DT == f16 else np.float32
EXP = mybir.ActivationFunctionType.Exp
MULT = mybir.AluOpType.mult
ADD = mybir.AluOpType.add


def _install_profhook():
    """Register the NTFF profile hook so trace=True yields HW exec time."""
    if "antenv.axon_hooks" in sys.modules:
        return
    try:
        import antenv
        from trn_agent_boot.trn_boot import _ntff_profile_via_ctypes

        mod = types.ModuleType("antenv.axon_hooks")
        state = {"hook": _ntff_profile_via_ctypes("/opt/axon/libaxon_pjrt.so")}
        mod.set_axon_ntff_profile_hook = lambda h: state.update(hook=h)
        mod.get_axon_ntff_profile_hook = lambda: state["hook"]
        sys.modules["antenv.axon_hooks"] = mod
        antenv.axon_hooks = mod
    except Exception:
        pass


def build_program():
    nc = bacc.Bacc("TRN2", target_bir_lowering=False, debug=False, num_devices=N_CORES)

    xt_d = nc.dram_tensor("xt", [D, S], MM_DT, kind="ExternalInput").ap()
    wq_d = nc.dram_tensor("wq", [D, NH * HD], MM_DT, kind="ExternalInput").ap()
    wk_d = nc.dram_tensor("wk", [D, NKV * HD], MM_DT, kind="ExternalInput").ap()
    wv_d = nc.dram_tensor("wv", [D, NKV * HD], MM_DT, kind="ExternalInput").ap()
    wo_d = nc.dram_tensor("wo", [NH * HD, D], MM_DT, kind="ExternalInput").ap()
    cb_d = nc.dram_tensor("cb", [128, S], f32, kind="ExternalInput").ap()
    sbc_d = nc.dram_tensor("sbc", [128, S], f32, kind="ExternalInput").ap()
    maskd_d = nc.dram_tensor("maskd", [128, 128], MM_DT, kind="ExternalInput").ap()
    ones_d = nc.dram_tensor("ones8", [128, 8], MM_DT, kind="ExternalInput").ap()
    out_d = nc.dram_tensor("out", [S, D], f32, kind="ExternalOutput").ap()

    with tile.TileContext(nc) as tc, ExitStack() as top:
        pers = top.enter_context(tc.tile_pool(name="pers", bufs=1))
        qT = [pers.tile([128, S], MM_DT, tag=f"qT{m}", name=f"qT{m}") for m in range(8)]
        kdup = [pers.tile([128, S], MM_DT, tag=f"kdup{t}", name=f"kdup{t}") for t in range(8)]
        vstore = [pers.tile([128, 8 * 65], MM_DT, tag=f"vst{s}", name=f"vst{s}") for s in range(8)]
        cb_sb = pers.tile([128, S], f32, tag="cb")
        sb_sb = pers.tile([128, S], f32, tag="sbc")
        maskb_sb = pers.tile([128, 128], MM_DT, tag="maskb")

        nc.sync.dma_start(cb_sb[:], cb_d)
        nc.sync.dma_start(sb_sb[:], sbc_d)
        nc.sync.dma_start(maskb_sb[:], maskd_d)

        with tc.tile_pool(name="xtp", bufs=1) as xtp:
            xt_sb = [xtp.tile([128, S], MM_DT, tag=f"xt{d}", name=f"xt{d}") for d in range(8)]
            for d in range(8):
                nc.sync.dma_start(
                    xt_sb[d][:], xt_d[d * 128 : (d + 1) * 128, :]
                )

            # ---- Phase 1a: v projection -> vstore (with ones col per head) ----
            with (
                tc.tile_pool(name="wvp", bufs=1) as wvp,
                tc.tile_pool(name="ps1a", bufs=2, space="PSUM") as ps1a,
            ):
                wv_sb = [wvp.tile([128, 512], MM_DT, tag=f"wv{d}", name=f"wv{d}") for d in range(8)]
                for d in range(8):
                    nc.sync.dma_start(
                        wv_sb[d][:], wv_d[d * 128 : (d + 1) * 128, :]
                    )
                for s in range(8):
                    ps = ps1a.tile([128, 512], f32, tag="vps")
                    for d in range(8):
                        nc.tensor.matmul(
                            ps[:],
                            xt_sb[d][:, s * 128 : (s + 1) * 128],
                            wv_sb[d][:],
                            start=(d == 0),
                            stop=(d == 7),
                        )
                    v3 = vstore[s][:].rearrange("p (g c) -> p g c", c=65)
                    nc.vector.tensor_copy(
                        v3[:, :, 0:64], ps[:].rearrange("p (g c) -> p g c", c=64)
                    )
                    nc.sync.dma_start(
                        v3[:, :, 64:65],
                        ones_d[:].rearrange("p (g o) -> p g o", o=1),
                    )

            # ---- Phase 1b/1c: k then q projections + RoPE ----
            with (
                tc.tile_pool(name="projp", bufs=3) as projp,
                tc.tile_pool(name="ktmpp", bufs=2) as ktmpp,
                tc.tile_pool(name="ps1b", bufs=3, space="PSUM") as ps1b,
            ):

                def project(m, w_d, dest_tile):
                    """dest_tile[:, :] = rope( w_d[:, m*128:+128].T @ xT )"""
                    wslice = projp.tile([128, 1024], MM_DT, tag="wslice")
                    nc.sync.dma_start(
                        wslice[:].rearrange("p (d c) -> p d c", c=128),
                        w_d[:, m * 128 : (m + 1) * 128]
                        .rearrange("(d p) c -> p d c", p=128)
                        ,
                    )
                    pqs = [
                        ps1b.tile([128, 512], f32, tag=f"pq{s}", name=f"pq{s}")
                        for s in range(2)
                    ]
                    for d in range(8):
                        lhs = wslice[:, d * 128 : (d + 1) * 128]
                        for s in range(2):
                            nc.tensor.matmul(
                                pqs[s][:],
                                lhs,
                                xt_sb[d][:, s * 512 : (s + 1) * 512],
                                start=(d == 0),
                                stop=(d == 7),
                            )
                    # rope on the full [128, 1024] row block
                    qn = projp.tile([128, S], f32, tag="qn")
                    for s in range(2):
                        nc.scalar.copy(qn[:, s * 512 : (s + 1) * 512], pqs[s][:])
                    qsw = projp.tile([128, S], f32, tag="qsw")
                    for b0 in (0, 64):
                        nc.sync.dma_start(
                            qsw[b0 : b0 + 32, :], qn[b0 + 32 : b0 + 64, :]
                        )
                        nc.sync.dma_start(
                            qsw[b0 + 32 : b0 + 64, :], qn[b0 : b0 + 32, :]
                        )
                    nc.vector.tensor_tensor(qsw[:], qsw[:], sb_sb[:], MULT)
                    nc.vector.tensor_tensor(qn[:], qn[:], cb_sb[:], MULT)
                    nc.vector.tensor_tensor(dest_tile[:], qn[:], qsw[:], ADD)

                for m in range(4):
                    ktmp = ktmpp.tile([128, S], MM_DT, tag="ktmp")
                    project(m, wk_d, ktmp)
                    for half in range(2):
                        src = ktmp[half * 64 : half * 64 + 64, :]
                        kd = kdup[2 * m + half]
                        nc.sync.dma_start(kd[0:64, :], src)
                        nc.sync.dma_start(kd[64:128, :], src)
                for m in range(8):
                    project(m, wq_d, qT[m])

        # ---- Phase 2: attention ----
        aotp = top.enter_context(tc.tile_pool(name="aotp", bufs=1))
        aoT = [aotp.tile([128, S], MM_DT, tag=f"aoT{m}", name=f"aoT{m}") for m in range(8)]
        with (
            tc.tile_pool(name="attp", bufs=3) as attp,
            tc.tile_pool(name="nrmp", bufs=2) as nrmp,
            tc.tile_pool(name="ps2s", bufs=2, space="PSUM") as ps2s,
            tc.tile_pool(name="ps2o", bufs=2, space="PSUM") as ps2o,
        ):
            for t in range(8):
                for c in range(2):
                    kjs = list(range(4 if c == 0 else 8))
                    po = ps2o.tile([65, 1024], f32, tag="po")  # A bank | B bank

                    def scores_exp(kj):
                        qlo = max(kj * 128, c * 512)
                        N = (c + 1) * 512 - qlo
                        sc = ps2s.tile([128, 1024], f32, tag="sc", name="sc")
                        for h in range(2):
                            nc.tensor.matmul(
                                sc[:, h * 512 : h * 512 + N],
                                kdup[t][h * 64 : h * 64 + 64, kj * 128 : (kj + 1) * 128],
                                qT[t][h * 64 : h * 64 + 64, qlo : qlo + N],
                                start=True,
                                stop=True,
                                tile_position=(h * 64, 0),
                            )
                        p_ = attp.tile([128, 1024], MM_DT, tag="p", name="p")
                        p3 = p_[:].rearrange("p (h n) -> p h n", h=2)
                        s3 = sc[:].rearrange("p (h n) -> p h n", h=2)
                        nc.scalar.activation(p3[:, :, 0:N], s3[:, :, 0:N], EXP)
                        if qlo == kj * 128:
                            for h in range(2):
                                nc.vector.tensor_tensor(
                                    p_[:, h * 512 : h * 512 + 128],
                                    p_[:, h * 512 : h * 512 + 128],
                                    maskb_sb[:],
                                    MULT,
                                )
                        return p_, qlo, N

                    def vmms(kj, p_, qlo, N):
                        off = qlo - c * 512
                        vh = vstore[kj][:, t * 65 : t * 65 + 65]
                        for h in range(2):
                            nc.tensor.matmul(
                                po[0:65, h * 512 + off : h * 512 + off + N],
                                vh,
                                p_[:, h * 512 : h * 512 + N],
                                start=(kj == 0),
                                stop=(kj == kjs[-1]),
                                skip_group_check=True,
                            )

                    pending = None
                    for kj in kjs:
                        se = scores_exp(kj)
                        if pending is not None:
                            vmms(*pending)
                        pending = (kj, *se) and (kj, se[0], se[1], se[2])
                    vmms(*pending)
                    rec = nrmp.tile([128, S], f32, tag="rec")
                    nc.scalar.copy(rec[64:65, :], po[64:65, :])
                    nc.sync.dma_start(rec[0:1, :], rec[64:65, :])
                    nc.vector.reciprocal_approx_fast(rec[0:1, :], rec[0:1, :])
                    bc = nrmp.tile([128, S], f32, tag="bc")
                    nc.gpsimd.partition_broadcast(bc[0:64, :], rec[0:1, :])
                    nc.vector.tensor_tensor(
                        aoT[t][0:64, c * 512 : (c + 1) * 512],
                        po[0:64, 0:512],
                        bc[0:64, 0:512],
                        MULT,
                    )
                    tmpB = nrmp.tile([128, 512], MM_DT, tag="tmpB")
                    nc.vector.tensor_tensor(
                        tmpB[0:64, :], po[0:64, 512:1024], bc[0:64, 512:1024], MULT
                    )
                    nc.sync.dma_start(
                        aoT[t][64:128, c * 512 : (c + 1) * 512], tmpB[0:64, :]
                    )

        # ---- Phase 3: output projection ----
        with (
            tc.tile_pool(name="wop", bufs=1) as wop,
            tc.tile_pool(name="ostp", bufs=3) as ostp,
            tc.tile_pool(name="ps3", bufs=2, space="PSUM") as ps3,
        ):
            wo_sb = [wop.tile([128, 1024], MM_DT, tag=f"wo{m}", name=f"wo{m}") for m in range(8)]
            for m in range(8):
                nc.sync.dma_start(
                    wo_sb[m][:], wo_d[m * 128 : (m + 1) * 128, :]
                )
            for st in range(8):
                po = [ps3.tile([128, 512], f32, tag=f"fo{j}", name=f"fo{j}") for j in range(2)]
                for m in range(8):
                    lhs = aoT[m][:, st * 128 : (st + 1) * 128]
                    for j in range(2):
                        nc.tensor.matmul(
                            po[j][:],
                            lhs,
                            wo_sb[m][:, j * 512 : (j + 1) * 512],
                            start=(m == 0),
                            stop=(m == 7),
                        )
                ob = ostp.tile([128, 1024], f32, tag="ob")
                for j in range(2):
                    nc.scalar.copy(ob[:, j * 512 : (j + 1) * 512], po[j][:])
                nc.sync.dma_start(out_d[st * 128 : (st + 1) * 128, :], ob[:])

    nc.compile()
    return nc


_NC = None


def _get_nc():
    global _NC
    if _NC is None:
        _NC = build_program()
    return _NC


def _host_prep(x, wq, wk, wv, wo, fcc, fcs):
    perm64 = np.concatenate([np.arange(0, 64, 2), np.arange(1, 64, 2)])
    perm_q = np.concatenate([h * 64 + perm64 for h in range(NH)])
    perm_k = np.concatenate([h * 64 + perm64 for h in range(NKV)])
    wq_p = np.ascontiguousarray(wq[:, perm_q], dtype=np.float32)
    wk_p = np.ascontiguousarray(wk[:, perm_k], dtype=np.float32)
    cb = (np.tile(fcc.T, (4, 1)) * ALPHA).astype(np.float32)
    sgn = np.where(np.arange(128) % 64 < 32, -1.0, 1.0).astype(np.float32)
    sbc = (np.tile(fcs.T, (4, 1)) * ALPHA * sgn[:, None]).astype(np.float32)
    jj = np.arange(128)[:, None] // BLK
    ii = np.arange(128)[None, :] // BLK
    maskd = np.where(ii >= jj, 1.0, 0.0).astype(np.float32)
    ones8 = np.ones((128, 8), np.float32)
    shared = {
        "wq": wq_p.astype(MM_NP),
        "wk": wk_p.astype(MM_NP),
        "wv": np.ascontiguousarray(wv).astype(MM_NP),
        "wo": np.ascontiguousarray(wo).astype(MM_NP),
        "cb": cb,
        "sbc": sbc,
        "maskd": maskd.astype(MM_NP),
        "ones8": ones8.astype(MM_NP),
    }
    in_maps = []
    for b in range(B):
        m = dict(shared)
        m["xt"] = np.ascontiguousarray(x[b].T).astype(MM_NP)
        in_maps.append(m)
    return in_maps


def kernel(x, wq, wk, wv, wo, freqs_cis_cos, freqs_cis_sin, start_pos=0):
    _install_profhook()
    x = np.asarray(x, dtype=np.float32)
    in_maps = _host_prep(
        x,
        np.asarray(wq, dtype=np.float32),
        np.asarray(wk, dtype=np.float32),
        np.asarray(wv, dtype=np.float32),
        np.asarray(wo, dtype=np.float32),
        np.asarray(freqs_cis_cos, dtype=np.float32),
        np.asarray(freqs_cis_sin, dtype=np.float32),
    )
    nc = _get_nc()
    trace = bool(int(os.environ.get("KERNEL_TRACE", "0")))
    res = run_bass_kernel_spmd(
        nc, in_maps, core_ids=list(range(N_CORES)), trace=trace
    )
    if trace:
        kernel.last_exec_time_ns = res.exec_time_ns
        kernel.last_result = res
    out = np.stack([res.results[c]["out"] for c in range(N_CORES)])
    return out


kernel.last_exec_time_ns = None
kernel.last_result = None


# revision 15
# speedup vs baseline: 1.4137x; 1.0088x over previous
"""Trainium2 Bass kernel for block-causal GQA attention (B=8,S=1024,D=1024,NH=16,NKV=8,HD=64,BLK=8).

Strategy: pure data-parallel over batch (1 batch element per NeuronCore, 8 cores).
Per core, everything is computed in a transposed ("T") layout so no on-device
transposes of activations or probabilities are ever needed:
  - host feeds x[b].T;  qT/kT are projected directly as  w.T @ x.T  (lhsT=w, rhs=xT)
  - RoPE is applied in the T layout using de-interleaved head dims (weight columns
    are permuted on the host; a 32-row block swap via SBUF-SBUF DMA supplies the
    rotated partner), with the 1/sqrt(HD) score scale folded into the cos/sin tables
  - scores are computed directly transposed:  sT[j,i] = k_tile.T @ qT  with two
    heads per PE pass (GQA pair shares the kv head; row-tiled K=64 matmuls at
    tile_position (0,0)/(64,0) run concurrently into the two banks of one
    2-bank PSUM tile, so softmax exp / reciprocal / broadcast run once per pair)
  - softmax denominator comes free from a ones-column appended to v (M=65 matmul);
    normalization happens on the 64xS per-head output, not on the SxS probabilities
  - attn-out is produced transposed, which is exactly the lhsT layout the final
    wo projection needs.
Matmuls run in float32r (full PE rate at N>=256).
"""

import os
import sys
import types
import math
import numpy as np
from contextlib import ExitStack

for _p in ("/opt/trn_rl_repo", "/root/.axon_site/_ro/trn_rl_repo"):
    if os.path.isdir(_p) and _p not in sys.path:
        sys.path.insert(0, _p)

import concourse.bass as bass
import concourse.tile as tile
from concourse import bacc, mybir
from concourse.bass_utils import run_bass_kernel_spmd

B, S, D = 8, 1024, 1024
NH, NKV, HD = 16, 8, 64
BLK = 8
N_CORES = 8
ALPHA = (1.0 / math.sqrt(HD)) ** 0.5

f32 = mybir.dt.float32
f32r = mybir.dt.float32r
f16 = mybir.dt.float16
MM_DT = f16 if os.environ.get("KERNEL_MM_DT", "f16") == "f16" else f32r
MM_NP = np.float16 if MM_DT == f16 else np.float32
EXP = mybir.ActivationFunctionType.Exp
MULT = mybir.AluOpType.mult
ADD = mybir.AluOpType.add


def _install_profhook():
    """Register the NTFF profile hook so trace=True yields HW exec time."""
    if "antenv.axon_hooks" in sys.modules:
        return
    try:
        import antenv
        from trn_agent_boot.trn_boot import _ntff_profile_via_ctypes

        mod = types.ModuleType("antenv.axon_hooks")
        state = {"hook": _ntff_profile_via_ctypes("/opt/axon/libaxon_pjrt.so")}
        mod.set_axon_ntff_profile_hook = lambda h: state.update(hook=h)
        mod.get_axon_ntff_profile_hook = lambda: state["hook"]
        sys.modules["antenv.axon_hooks"] = mod
        antenv.axon_hooks = mod
    except Exception:
        pass


def build_program():
    nc = bacc.Bacc("TRN2", target_bir_lowering=False, debug=False, num_devices=N_CORES)

    xt_d = nc.dram_tensor("xt", [D, S], MM_DT, kind="ExternalInput").ap()
    wq_d = nc.dram_tensor("wq", [D, NH * HD], MM_DT, kind="ExternalInput").ap()
    wk_d = nc.dram_tensor("wk", [D, NKV * HD], MM_DT, kind="ExternalInput").ap()
    wv_d = nc.dram_tensor("wv", [D, NKV * HD], MM_DT, kind="ExternalInput").ap()
    wo_d = nc.dram_tensor("wo", [NH * HD, D], MM_DT, kind="ExternalInput").ap()
    cb_d = nc.dram_tensor("cb", [128, S], f32, kind="ExternalInput").ap()
    sbc_d = nc.dram_tensor("sbc", [128, S], f32, kind="ExternalInput").ap()
    maskd_d = nc.dram_tensor("maskd", [128, 128], MM_DT, kind="ExternalInput").ap()
    ones_d = nc.dram_tensor("ones8", [128, 8], MM_DT, kind="ExternalInput").ap()
    out_d = nc.dram_tensor("out", [S, D], f32, kind="ExternalOutput").ap()

    with tile.TileContext(nc) as tc, ExitStack() as top:
        pers = top.enter_context(tc.tile_pool(name="pers", bufs=1))
        qT = [pers.tile([128, S], MM_DT, tag=f"qT{m}", name=f"qT{m}") for m in range(8)]
        kdup = [pers.tile([128, S], MM_DT, tag=f"kdup{t}", name=f"kdup{t}") for t in range(8)]
        vstore = [pers.tile([128, 8 * 65], MM_DT, tag=f"vst{s}", name=f"vst{s}") for s in range(8)]
        cb_sb = pers.tile([128, S], f32, tag="cb")
        sb_sb = pers.tile([128, S], f32, tag="sbc")
        maskb_sb = pers.tile([128, 128], MM_DT, tag="maskb")

        nc.sync.dma_start(cb_sb[:], cb_d)
        nc.sync.dma_start(sb_sb[:], sbc_d)
        nc.sync.dma_start(maskb_sb[:], maskd_d)

        with tc.tile_pool(name="xtp", bufs=1) as xtp:
            xt_sb = [xtp.tile([128, S], MM_DT, tag=f"xt{d}", name=f"xt{d}") for d in range(8)]
            for d in range(8):
                nc.sync.dma_start(
                    xt_sb[d][:], xt_d[d * 128 : (d + 1) * 128, :]
                )

            # ---- Phase 1a: v projection -> vstore (with ones col per head) ----
            with (
                tc.tile_pool(name="wvp", bufs=1) as wvp,
                tc.tile_pool(name="ps1a", bufs=2, space="PSUM") as ps1a,
            ):
                wv_sb = [wvp.tile([128, 512], MM_DT, tag=f"wv{d}", name=f"wv{d}") for d in range(8)]
                for d in range(8):
                    nc.sync.dma_start(
                        wv_sb[d][:], wv_d[d * 128 : (d + 1) * 128, :]
                    )
                for s in range(8):
                    ps = ps1a.tile([128, 512], f32, tag="vps")
                    for d in range(8):
                        nc.tensor.matmul(
                            ps[:],
                            xt_sb[d][:, s * 128 : (s + 1) * 128],
                            wv_sb[d][:],
                            start=(d == 0),
                            stop=(d == 7),
                        )
                    v3 = vstore[s][:].rearrange("p (g c) -> p g c", c=65)
                    nc.vector.tensor_copy(
                        v3[:, :, 0:64], ps[:].rearrange("p (g c) -> p g c", c=64)
                    )
                    nc.sync.dma_start(
                        v3[:, :, 64:65],
                        ones_d[:].rearrange("p (g o) -> p g o", o=1),
                    )

            # ---- Phase 1b/1c: k then q projections + RoPE ----
            with (
                tc.tile_pool(name="projp", bufs=3) as projp,
                tc.tile_pool(name="ktmpp", bufs=2) as ktmpp,
                tc.tile_pool(name="ps1b", bufs=3, space="PSUM") as ps1b,
            ):

                def project(m, w_d, dest_tile):
                    """dest_tile[:, :] = rope( w_d[:, m*128:+128].T @ xT )"""
                    wslice = projp.tile([128, 1024], MM_DT, tag="wslice")
                    nc.sync.dma_start(
                        wslice[:].rearrange("p (d c) -> p d c", c=128),
                        w_d[:, m * 128 : (m + 1) * 128]
                        .rearrange("(d p) c -> p d c", p=128)
                        ,
                    )
                    pqs = [
                        ps1b.tile([128, 512], f32, tag=f"pq{s}", name=f"pq{s}")
                        for s in range(2)
                    ]
                    for d in range(8):
                        lhs = wslice[:, d * 128 : (d + 1) * 128]
                        for s in range(2):
                            nc.tensor.matmul(
                                pqs[s][:],
                                lhs,
                                xt_sb[d][:, s * 512 : (s + 1) * 512],
                                start=(d == 0),
                                stop=(d == 7),
                            )
                    # rope on the full [128, 1024] row block
                    qn = projp.tile([128, S], f32, tag="qn")
                    for s in range(2):
                        nc.scalar.copy(qn[:, s * 512 : (s + 1) * 512], pqs[s][:])
                    qsw = projp.tile([128, S], f32, tag="qsw")
                    for b0 in (0, 64):
                        nc.sync.dma_start(
                            qsw[b0 : b0 + 32, :], qn[b0 + 32 : b0 + 64, :]
                        )
                        nc.sync.dma_start(
                            qsw[b0 + 32 : b0 + 64, :], qn[b0 : b0 + 32, :]
                        )
                    nc.vector.tensor_tensor(qsw[:], qsw[:], sb_sb[:], MULT)
                    nc.vector.tensor_tensor(qn[:], qn[:], cb_sb[:], MULT)
                    nc.vector.tensor_tensor(dest_tile[:], qn[:], qsw[:], ADD)

                for m in range(4):
                    ktmp = ktmpp.tile([128, S], MM_DT, tag="ktmp")
                    project(m, wk_d, ktmp)
                    for half in range(2):
                        src = ktmp[half * 64 : half * 64 + 64, :]
                        kd = kdup[2 * m + half]
                        nc.sync.dma_start(kd[0:64, :], src)
                        nc.sync.dma_start(kd[64:128, :], src)
                for m in range(8):
                    project(m, wq_d, qT[m])

        # ---- Phase 2: attention ----
        aotp = top.enter_context(tc.tile_pool(name="aotp", bufs=1))
        aoT = [aotp.tile([128, S], MM_DT, tag=f"aoT{m}", name=f"aoT{m}") for m in range(8)]
        with (
            tc.tile_pool(name="attp", bufs=3) as attp,
            tc.tile_pool(name="nrmp", bufs=2) as nrmp,
            tc.tile_pool(name="ps2s", bufs=2, space="PSUM") as ps2s,
            tc.tile_pool(name="ps2o", bufs=2, space="PSUM") as ps2o,
        ):
            for t in range(8):
                for c in range(2):
                    kjs = list(range(4 if c == 0 else 8))
                    po = ps2o.tile([65, 1024], f32, tag="po")  # A bank | B bank

                    def scores_exp(kj):
                        qlo = max(kj * 128, c * 512)
                        N = (c + 1) * 512 - qlo
                        sc = ps2s.tile([128, 1024], f32, tag="sc", name="sc")
                        for h in range(2):
                            nc.tensor.matmul(
                                sc[:, h * 512 : h * 512 + N],
                                kdup[t][h * 64 : h * 64 + 64, kj * 128 : (kj + 1) * 128],
                                qT[t][h * 64 : h * 64 + 64, qlo : qlo + N],
                                start=True,
                                stop=True,
                                tile_position=(h * 64, 0),
                            )
                        p_ = attp.tile([128, 1024], MM_DT, tag="p", name="p")
                        p3 = p_[:].rearrange("p (h n) -> p h n", h=2)
                        s3 = sc[:].rearrange("p (h n) -> p h n", h=2)
                        nc.scalar.activation(p3[:, :, 0:N], s3[:, :, 0:N], EXP)
                        if qlo == kj * 128:
                            for h in range(2):
                                nc.vector.tensor_tensor(
                                    p_[:, h * 512 : h * 512 + 128],
                                    p_[:, h * 512 : h * 512 + 128],
                                    maskb_sb[:],
                                    MULT,
                                )
                        return p_, qlo, N

                    def vmms(kj, p_, qlo, N):
                        off = qlo - c * 512
                        vh = vstore[kj][:, t * 65 : t * 65 + 65]
                        for h in range(2):
                            nc.tensor.matmul(
                                po[0:65, h * 512 + off : h * 512 + off + N],
                                vh,
                                p_[:, h * 512 : h * 512 + N],
                                start=(kj == 0),
                                stop=(kj == kjs[-1]),
                                skip_group_check=True,
                            )

                    pending = None
                    for kj in kjs:
                        se = scores_exp(kj)
                        if pending is not None:
                            vmms(*pending)
                        pending = (kj, *se) and (kj, se[0], se[1], se[2])
                    vmms(*pending)
                    rec = nrmp.tile([128, S], f32, tag="rec")
                    nc.scalar.copy(rec[64:65, :], po[64:65, :])
                    nc.sync.dma_start(rec[0:1, :], rec[64:65, :])
                    nc.vector.reciprocal_approx_fast(rec[0:1, :], rec[0:1, :])
                    bc = nrmp.tile([128, S], f32, tag="bc")
                    nc.gpsimd.partition_broadcast(bc[0:64, :], rec[0:1, :])
                    nc.vector.tensor_tensor(
                        aoT[t][0:64, c * 512 : (c + 1) * 512],
                        po[0:64, 0:512],
                        bc[0:64, 0:512],
                        MULT,
                    )
                    tmpB = nrmp.tile([128, 512], MM_DT, tag="tmpB")
                    nc.vector.tensor_tensor(
                        tmpB[0:64, :], po[0:64, 512:1024], bc[0:64, 512:1024], MULT
                    )
                    nc.sync.dma_start(
                        aoT[t][64:128, c * 512 : (c + 1) * 512], tmpB[0:64, :]
                    )

        # ---- Phase 3: output projection ----
        with (
            tc.tile_pool(name="wop", bufs=1) as wop,
            tc.tile_pool(name="ostp", bufs=3) as ostp,
            tc.tile_pool(name="ps3", bufs=2, space="PSUM") as ps3,
        ):
            wo_sb = [wop.tile([128, 1024], MM_DT, tag=f"wo{m}", name=f"wo{m}") for m in range(8)]
            for m in range(8):
                nc.sync.dma_start(
                    wo_sb[m][:], wo_d[m * 128 : (m + 1) * 128, :]
                )
            for st in range(8):
                po = [ps3.tile([128, 512], f32, tag=f"fo{j}", name=f"fo{j}") for j in range(2)]
                for m in range(8):
                    lhs = aoT[m][:, st * 128 : (st + 1) * 128]
                    for j in range(2):
                        nc.tensor.matmul(
                            po[j][:],
                            lhs,
                            wo_sb[m][:, j * 512 : (j + 1) * 512],
                            start=(m == 0),
                            stop=(m == 7),
                        )
                ob = ostp.tile([128, 1024], f32, tag="ob")
                for j in range(2):
                    nc.vector.tensor_copy(ob[:, j * 512 : (j + 1) * 512], po[j][:])
                nc.sync.dma_start(out_d[st * 128 : (st + 1) * 128, :], ob[:])

    nc.compile()
    return nc


_NC = None


def _get_nc():
    global _NC
    if _NC is None:
        _NC = build_program()
    return _NC


def _host_prep(x, wq, wk, wv, wo, fcc, fcs):
    perm64 = np.concatenate([np.arange(0, 64, 2), np.arange(1, 64, 2)])
    perm_q = np.concatenate([h * 64 + perm64 for h in range(NH)])
    perm_k = np.concatenate([h * 64 + perm64 for h in range(NKV)])
    wq_p = np.ascontiguousarray(wq[:, perm_q], dtype=np.float32)
    wk_p = np.ascontiguousarray(wk[:, perm_k], dtype=np.float32)
    cb = (np.tile(fcc.T, (4, 1)) * ALPHA).astype(np.float32)
    sgn = np.where(np.arange(128) % 64 < 32, -1.0, 1.0).astype(np.float32)
    sbc = (np.tile(fcs.T, (4, 1)) * ALPHA * sgn[:, None]).astype(np.float32)
    jj = np.arange(128)[:, None] // BLK
    ii = np.arange(128)[None, :] // BLK
    maskd = np.where(ii >= jj, 1.0, 0.0).astype(np.float32)
    ones8 = np.ones((128, 8), np.float32)
    shared = {
        "wq": wq_p.astype(MM_NP),
        "wk": wk_p.astype(MM_NP),
        "wv": np.ascontiguousarray(wv).astype(MM_NP),
        "wo": np.ascontiguousarray(wo).astype(MM_NP),
        "cb": cb,
        "sbc": sbc,
        "maskd": maskd.astype(MM_NP),
        "ones8": ones8.astype(MM_NP),
    }
    in_maps = []
    for b in range(B):
        m = dict(shared)
        m["xt"] = np.ascontiguousarray(x[b].T).astype(MM_NP)
        in_maps.append(m)
    return in_maps


def kernel(x, wq, wk, wv, wo, freqs_cis_cos, freqs_cis_sin, start_pos=0):
    _install_profhook()
    x = np.asarray(x, dtype=np.float32)
    in_maps = _host_prep(
        x,
        np.asarray(wq, dtype=np.float32),
        np.asarray(wk, dtype=np.float32),
        np.asarray(wv, dtype=np.float32),
        np.asarray(wo, dtype=np.float32),
        np.asarray(freqs_cis_cos, dtype=np.float32),
        np.asarray(freqs_cis_sin, dtype=np.float32),
    )
    nc = _get_nc()
    trace = bool(int(os.environ.get("KERNEL_TRACE", "0")))
    res = run_bass_kernel_spmd(
        nc, in_maps, core_ids=list(range(N_CORES)), trace=trace
    )
    if trace:
        kernel.last_exec_time_ns = res.exec_time_ns
        kernel.last_result = res
    out = np.stack([res.results[c]["out"] for c in range(N_CORES)])
    return out


kernel.last_exec_time_ns = None
kernel.last_result = None


# revision 17
# speedup vs baseline: 1.4239x; 1.0072x over previous
"""Trainium2 Bass kernel for block-causal GQA attention (B=8,S=1024,D=1024,NH=16,NKV=8,HD=64,BLK=8).

Strategy: pure data-parallel over batch (1 batch element per NeuronCore, 8 cores).
Per core, everything is computed in a transposed ("T") layout so no on-device
transposes of activations or probabilities are ever needed:
  - host feeds x[b].T;  qT/kT are projected directly as  w.T @ x.T  (lhsT=w, rhs=xT)
  - RoPE is applied in the T layout using de-interleaved head dims (weight columns
    are permuted on the host; a 32-row block swap via SBUF-SBUF DMA supplies the
    rotated partner), with the 1/sqrt(HD) score scale folded into the cos/sin tables
  - scores are computed directly transposed:  sT[j,i] = k_tile.T @ qT  with two
    heads per PE pass (GQA pair shares the kv head; row-tiled K=64 matmuls at
    tile_position (0,0)/(64,0) run concurrently into the two banks of one
    2-bank PSUM tile, so softmax exp / reciprocal / broadcast run once per pair)
  - softmax denominator comes free from a ones-column appended to v (M=65 matmul);
    normalization happens on the 64xS per-head output, not on the SxS probabilities
  - attn-out is produced transposed, which is exactly the lhsT layout the final
    wo projection needs.
Matmuls run in float32r (full PE rate at N>=256).
"""

import os
import sys
import types
import math
import numpy as np
from contextlib import ExitStack

for _p in ("/opt/trn_rl_repo", "/root/.axon_site/_ro/trn_rl_repo"):
    if os.path.isdir(_p) and _p not in sys.path:
        sys.path.insert(0, _p)

import concourse.bass as bass
import concourse.tile as tile
from concourse import bacc, mybir
from concourse.bass_utils import run_bass_kernel_spmd

B, S, D = 8, 1024, 1024
NH, NKV, HD = 16, 8, 64
BLK = 8
N_CORES = 8
ALPHA = (1.0 / math.sqrt(HD)) ** 0.5

f32 = mybir.dt.float32
f32r = mybir.dt.float32r
f16 = mybir.dt.float16
MM_DT = f16 if os.environ.get("KERNEL_MM_DT", "f16") == "f16" else f32r
MM_NP = np.float16 if MM_DT == f16 else np.float32
EXP = mybir.ActivationFunctionType.Exp
MULT = mybir.AluOpType.mult
ADD = mybir.AluOpType.add


def _install_profhook():
    """Register the NTFF profile hook so trace=True yields HW exec time."""
    if "antenv.axon_hooks" in sys.modules:
        return
    try:
        import antenv
        from trn_agent_boot.trn_boot import _ntff_profile_via_ctypes

        mod = types.ModuleType("antenv.axon_hooks")
        state = {"hook": _ntff_profile_via_ctypes("/opt/axon/libaxon_pjrt.so")}
        mod.set_axon_ntff_profile_hook = lambda h: state.update(hook=h)
        mod.get_axon_ntff_profile_hook = lambda: state["hook"]
        sys.modules["antenv.axon_hooks"] = mod
        antenv.axon_hooks = mod
    except Exception:
        pass


def build_program():
    nc = bacc.Bacc("TRN2", target_bir_lowering=False, debug=False, num_devices=N_CORES)

    xt_d = nc.dram_tensor("xt", [D, S], MM_DT, kind="ExternalInput").ap()
    wq_d = nc.dram_tensor("wq", [D, NH * HD], MM_DT, kind="ExternalInput").ap()
    wk_d = nc.dram_tensor("wk", [D, NKV * HD], MM_DT, kind="ExternalInput").ap()
    wv_d = nc.dram_tensor("wv", [D, NKV * HD], MM_DT, kind="ExternalInput").ap()
    wo_d = nc.dram_tensor("wo", [NH * HD, D], MM_DT, kind="ExternalInput").ap()
    cb_d = nc.dram_tensor("cb", [128, S], f32, kind="ExternalInput").ap()
    sbc_d = nc.dram_tensor("sbc", [128, S], f32, kind="ExternalInput").ap()
    maskd_d = nc.dram_tensor("maskd", [128, 128], MM_DT, kind="ExternalInput").ap()
    ones_d = nc.dram_tensor("ones8", [128, 8], MM_DT, kind="ExternalInput").ap()
    out_d = nc.dram_tensor("out", [S, D], f32, kind="ExternalOutput").ap()

    with tile.TileContext(nc) as tc, ExitStack() as top:
        pers = top.enter_context(tc.tile_pool(name="pers", bufs=1))
        qT = [pers.tile([128, S], MM_DT, tag=f"qT{m}", name=f"qT{m}") for m in range(8)]
        kdup = [pers.tile([128, S], MM_DT, tag=f"kdup{t}", name=f"kdup{t}") for t in range(8)]
        vstore = [pers.tile([128, 8 * 65], MM_DT, tag=f"vst{s}", name=f"vst{s}") for s in range(8)]
        cb_sb = pers.tile([128, S], f32, tag="cb")
        sb_sb = pers.tile([128, S], f32, tag="sbc")
        maskb_sb = pers.tile([128, 128], MM_DT, tag="maskb")

        nc.sync.dma_start(cb_sb[:], cb_d)
        nc.sync.dma_start(sb_sb[:], sbc_d)
        nc.sync.dma_start(maskb_sb[:], maskd_d)

        with tc.tile_pool(name="xtp", bufs=1) as xtp:
            xt_sb = [xtp.tile([128, S], MM_DT, tag=f"xt{d}", name=f"xt{d}") for d in range(8)]
            for d in range(8):
                nc.sync.dma_start(
                    xt_sb[d][:], xt_d[d * 128 : (d + 1) * 128, :]
                )

            # ---- Phase 1a: v projection -> vstore (with ones col per head) ----
            with (
                tc.tile_pool(name="wvp", bufs=1) as wvp,
                tc.tile_pool(name="ps1a", bufs=2, space="PSUM") as ps1a,
            ):
                wv_sb = [wvp.tile([128, 512], MM_DT, tag=f"wv{d}", name=f"wv{d}") for d in range(8)]
                for d in range(8):
                    nc.sync.dma_start(
                        wv_sb[d][:], wv_d[d * 128 : (d + 1) * 128, :]
                    )
                for s in range(8):
                    ps = ps1a.tile([128, 512], f32, tag="vps")
                    for d in range(8):
                        nc.tensor.matmul(
                            ps[:],
                            xt_sb[d][:, s * 128 : (s + 1) * 128],
                            wv_sb[d][:],
                            start=(d == 0),
                            stop=(d == 7),
                        )
                    v3 = vstore[s][:].rearrange("p (g c) -> p g c", c=65)
                    nc.vector.tensor_copy(
                        v3[:, :, 0:64], ps[:].rearrange("p (g c) -> p g c", c=64)
                    )
                    nc.sync.dma_start(
                        v3[:, :, 64:65],
                        ones_d[:].rearrange("p (g o) -> p g o", o=1),
                    )

            # ---- Phase 1b/1c: k then q projections + RoPE ----
            with (
                tc.tile_pool(name="projp", bufs=3) as projp,
                tc.tile_pool(name="ktmpp", bufs=2) as ktmpp,
                tc.tile_pool(name="ps1b", bufs=3, space="PSUM") as ps1b,
            ):

                def project(m, w_d, dest_tile):
                    """dest_tile[:, :] = rope( w_d[:, m*128:+128].T @ xT )"""
                    wslice = projp.tile([128, 1024], MM_DT, tag="wslice")
                    nc.sync.dma_start(
                        wslice[:].rearrange("p (d c) -> p d c", c=128),
                        w_d[:, m * 128 : (m + 1) * 128]
                        .rearrange("(d p) c -> p d c", p=128)
                        ,
                    )
                    pqs = [
                        ps1b.tile([128, 512], f32, tag=f"pq{s}", name=f"pq{s}")
                        for s in range(2)
                    ]
                    for d in range(8):
                        lhs = wslice[:, d * 128 : (d + 1) * 128]
                        for s in range(2):
                            nc.tensor.matmul(
                                pqs[s][:],
                                lhs,
                                xt_sb[d][:, s * 512 : (s + 1) * 512],
                                start=(d == 0),
                                stop=(d == 7),
                            )
                    # rope on the full [128, 1024] row block
                    qn = projp.tile([128, S], f32, tag="qn")
                    for s in range(2):
                        nc.scalar.copy(qn[:, s * 512 : (s + 1) * 512], pqs[s][:])
                    qsw = projp.tile([128, S], f32, tag="qsw")
                    for b0 in (0, 64):
                        nc.sync.dma_start(
                            qsw[b0 : b0 + 32, :], qn[b0 + 32 : b0 + 64, :]
                        )
                        nc.sync.dma_start(
                            qsw[b0 + 32 : b0 + 64, :], qn[b0 : b0 + 32, :]
                        )
                    nc.vector.tensor_tensor(qsw[:], qsw[:], sb_sb[:], MULT)
                    nc.vector.tensor_tensor(qn[:], qn[:], cb_sb[:], MULT)
                    nc.vector.tensor_tensor(dest_tile[:], qn[:], qsw[:], ADD)

                for m in range(4):
                    ktmp = ktmpp.tile([128, S], MM_DT, tag="ktmp")
                    project(m, wk_d, ktmp)
                    for half in range(2):
                        src = ktmp[half * 64 : half * 64 + 64, :]
                        kd = kdup[2 * m + half]
                        nc.sync.dma_start(kd[0:64, :], src)
                        nc.sync.dma_start(kd[64:128, :], src)
                for m in range(8):
                    project(m, wq_d, qT[m])

        # ---- Phase 2: attention ----
        aotp = top.enter_context(tc.tile_pool(name="aotp", bufs=1))
        aoT = [aotp.tile([128, S], MM_DT, tag=f"aoT{m}", name=f"aoT{m}") for m in range(8)]
        with (
            tc.tile_pool(name="attp", bufs=3) as attp,
            tc.tile_pool(name="nrmp", bufs=2) as nrmp,
            tc.tile_pool(name="ps2s", bufs=2, space="PSUM") as ps2s,
            tc.tile_pool(name="ps2o", bufs=2, space="PSUM") as ps2o,
        ):
            for t in range(8):
                for c in range(2):
                    kjs = list(range(4 if c == 0 else 8))
                    po = ps2o.tile([65, 1024], f32, tag="po")  # A bank | B bank

                    def scores_exp(kj):
                        qlo = max(kj * 128, c * 512)
                        N = (c + 1) * 512 - qlo
                        sc = ps2s.tile([128, 1024], f32, tag="sc", name="sc")
                        for h in range(2):
                            nc.tensor.matmul(
                                sc[:, h * 512 : h * 512 + N],
                                kdup[t][h * 64 : h * 64 + 64, kj * 128 : (kj + 1) * 128],
                                qT[t][h * 64 : h * 64 + 64, qlo : qlo + N],
                                start=True,
                                stop=True,
                                tile_position=(h * 64, 0),
                            )
                        p_ = attp.tile([128, 1024], MM_DT, tag="p", name="p")
                        p3 = p_[:].rearrange("p (h n) -> p h n", h=2)
                        s3 = sc[:].rearrange("p (h n) -> p h n", h=2)
                        nc.scalar.activation(p3[:, :, 0:N], s3[:, :, 0:N], EXP)
                        if qlo == kj * 128:
                            for h in range(2):
                                nc.vector.tensor_tensor(
                                    p_[:, h * 512 : h * 512 + 128],
                                    p_[:, h * 512 : h * 512 + 128],
                                    maskb_sb[:],
                                    MULT,
                                )
                        return p_, qlo, N

                    def vmms(kj, p_, qlo, N):
                        off = qlo - c * 512
                        vh = vstore[kj][:, t * 65 : t * 65 + 65]
                        for h in range(2):
                            nc.tensor.matmul(
                                po[0:65, h * 512 + off : h * 512 + off + N],
                                vh,
                                p_[:, h * 512 : h * 512 + N],
                                start=(kj == 0),
                                stop=(kj == kjs[-1]),
                                skip_group_check=True,
                            )

                    pending = None
                    for kj in kjs:
                        se = scores_exp(kj)
                        if pending is not None:
                            vmms(*pending)
                        pending = (kj, *se) and (kj, se[0], se[1], se[2])
                    vmms(*pending)
                    rec = nrmp.tile([128, S], f32, tag="rec")
                    nc.scalar.copy(rec[64:65, :], po[64:65, :])
                    nc.sync.dma_start(rec[0:1, :], rec[64:65, :])
                    rcp = nrmp.tile([128, S], f32, tag="rcp", name="rcp")
                    nc.vector.reciprocal_approx_fast(rcp[0:1, :], rec[0:1, :])
                    bc = nrmp.tile([128, S], f32, tag="bc")
                    nc.gpsimd.partition_broadcast(bc[0:64, :], rcp[0:1, :])
                    nc.vector.tensor_tensor(
                        aoT[t][0:64, c * 512 : (c + 1) * 512],
                        po[0:64, 0:512],
                        bc[0:64, 0:512],
                        MULT,
                    )
                    tmpB = nrmp.tile([128, 512], MM_DT, tag="tmpB")
                    nc.vector.tensor_tensor(
                        tmpB[0:64, :], po[0:64, 512:1024], bc[0:64, 512:1024], MULT
                    )
                    nc.sync.dma_start(
                        aoT[t][64:128, c * 512 : (c + 1) * 512], tmpB[0:64, :]
                    )

        # ---- Phase 3: output projection ----
        with (
            tc.tile_pool(name="wop", bufs=1) as wop,
            tc.tile_pool(name="ostp", bufs=3) as ostp,
            tc.tile_pool(name="ps3", bufs=2, space="PSUM") as ps3,
        ):
            wo_sb = [wop.tile([128, 1024], MM_DT, tag=f"wo{m}", name=f"wo{m}") for m in range(8)]
            for m in range(8):
                nc.sync.dma_start(
                    wo_sb[m][:], wo_d[m * 128 : (m + 1) * 128, :]
                )
            for st in range(8):
                po = [ps3.tile([128, 512], f32, tag=f"fo{j}", name=f"fo{j}") for j in range(2)]
                for m in range(8):
                    lhs = aoT[m][:, st * 128 : (st + 1) * 128]
                    for j in range(2):
                        nc.tensor.matmul(
                            po[j][:],
                            lhs,
                            wo_sb[m][:, j * 512 : (j + 1) * 512],
                            start=(m == 0),
                            stop=(m == 7),
                        )
                ob = ostp.tile([128, 1024], f32, tag="ob")
                for j in range(2):
                    nc.scalar.copy(ob[:, j * 512 : (j + 1) * 512], po[j][:])
                nc.sync.dma_start(out_d[st * 128 : (st + 1) * 128, :], ob[:])

    nc.compile()
    return nc


_NC = None


def _get_nc():
    global _NC
    if _NC is None:
        _NC = build_program()
    return _NC


def _host_prep(x, wq, wk, wv, wo, fcc, fcs):
    perm64 = np.concatenate([np.arange(0, 64, 2), np.arange(1, 64, 2)])
    perm_q = np.concatenate([h * 64 + perm64 for h in range(NH)])
    perm_k = np.concatenate([h * 64 + perm64 for h in range(NKV)])
    wq_p = np.ascontiguousarray(wq[:, perm_q], dtype=np.float32)
    wk_p = np.ascontiguousarray(wk[:, perm_k], dtype=np.float32)
    cb = (np.tile(fcc.T, (4, 1)) * ALPHA).astype(np.float32)
    sgn = np.where(np.arange(128) % 64 < 32, -1.0, 1.0).astype(np.float32)
    sbc = (np.tile(fcs.T, (4, 1)) * ALPHA * sgn[:, None]).astype(np.float32)
    jj = np.arange(128)[:, None] // BLK
    ii = np.arange(128)[None, :] // BLK
    maskd = np.where(ii >= jj, 1.0, 0.0).astype(np.float32)
    ones8 = np.ones((128, 8), np.float32)
    shared = {
        "wq": wq_p.astype(MM_NP),
        "wk": wk_p.astype(MM_NP),
        "wv": np.ascontiguousarray(wv).astype(MM_NP),
        "wo": np.ascontiguousarray(wo).astype(MM_NP),
        "cb": cb,
        "sbc": sbc,
        "maskd": maskd.astype(MM_NP),
        "ones8": ones8.astype(MM_NP),
    }
    in_maps = []
    for b in range(B):
        m = dict(shared)
        m["xt"] = np.ascontiguousarray(x[b].T).astype(MM_NP)
        in_maps.append(m)
    return in_maps


def kernel(x, wq, wk, wv, wo, freqs_cis_cos, freqs_cis_sin, start_pos=0):
    _install_profhook()
    x = np.asarray(x, dtype=np.float32)
    in_maps = _host_prep(
        x,
        np.asarray(wq, dtype=np.float32),
        np.asarray(wk, dtype=np.float32),
        np.asarray(wv, dtype=np.float32),
        np.asarray(wo, dtype=np.float32),
        np.asarray(freqs_cis_cos, dtype=np.float32),
        np.asarray(freqs_cis_sin, dtype=np.float32),
    )
    nc = _get_nc()
    trace = bool(int(os.environ.get("KERNEL_TRACE", "0")))
    res = run_bass_kernel_spmd(
        nc, in_maps, core_ids=list(range(N_CORES)), trace=trace
    )
    if trace:
        kernel.last_exec_time_ns = res.exec_time_ns
        kernel.last_result = res
    out = np.stack([res.results[c]["out"] for c in range(N_CORES)])
    return out


kernel.last_exec_time_ns = None
kernel.last_result = None
